# revision 1
# baseline (speedup 1.0000x reference)
"""GQA attention (B=2, T=2048, HID=2048, 32 q-heads / 8 kv-heads, d=64)
distributed over 8 TRN2 NeuronCores.

Sharding: tensor-parallel over heads. Core c owns q-heads [4c, 4c+4) and
kv-head c (column shards of Wq/Wk/Wv), plus the matching column shard of Wo
used to compute out^T rows. x is replicated (host pre-transposes to [hid, tok]
and casts to bf16). After local attention each core AllGathers its y^T
[256, 4096] block into the full y^T [2048, 4096], then computes
out^T[256c:256c+256, :] locally. The host concatenates and transposes.

All matmuls run in bf16 with f32 PSUM accumulation. Softmax runs without
max-subtraction (scores are O(10) for this distribution; exp is exact in f32)
and the denominator comes from a ones-row appended to V in the PV matmul.
"""

import os
import sys

import numpy as np

for _p in ("/opt/trn_rl_repo", "/root/.axon_site/_ro/trn_rl_repo"):
    if os.path.isdir(_p) and _p not in sys.path:
        sys.path.append(_p)

import ml_dtypes  # noqa: E402
from contextlib import ExitStack  # noqa: E402

import concourse.bass as bass  # noqa: E402
import concourse.tile as tile  # noqa: E402
from concourse import bacc, mybir  # noqa: E402
from concourse.bass_utils import run_bass_kernel_spmd  # noqa: E402

BF16 = mybir.dt.bfloat16
F32 = mybir.dt.float32
NPBF16 = ml_dtypes.bfloat16

B, T, HID = 2, 2048, 2048
NT = B * T
HEADS, KV_HEADS, DH = 32, 8, 64
NCORES = 8
QH = HEADS // NCORES          # q-heads per core
DQ = QH * DH                  # 256
KC = HID // 128               # 16 hidden-dim chunks
TC = T // 512                 # 4 token chunks of 512 per batch
JC = T // 128                 # 16 key chunks of 128 per batch
EXP = mybir.ActivationFunctionType.Exp


def _build(
    causal: bool,
    debug: bool = False,
    compile: bool = True,
    debug_dumps: bool = False,
) -> bacc.Bacc:
    nc = bacc.Bacc(
        "TRN2", target_bir_lowering=False, debug=debug, num_devices=NCORES
    )
    xT = nc.dram_tensor("xT", [B, KC, 128, T], BF16, kind="ExternalInput")
    wq = nc.dram_tensor("wq", [KC, 128, DQ], BF16, kind="ExternalInput")
    wkv = nc.dram_tensor("wkv", [KC, 128, 128], BF16, kind="ExternalInput")
    wo = nc.dram_tensor("wo", [KC, 128, DQ], BF16, kind="ExternalInput")
    mw = 512 if causal else T
    maskT = nc.dram_tensor("maskT", [JC, 128, mw], BF16, kind="ExternalInput")
    ident = nc.dram_tensor("ident", [128, 128], BF16, kind="ExternalInput")
    outT = nc.dram_tensor("outT", [2, 128, NT], F32, kind="ExternalOutput")
    if debug_dumps:
        dbg_q = nc.dram_tensor("dbg_q", [64, NT], BF16, kind="ExternalOutput")
        dbg_k = nc.dram_tensor("dbg_k", [64, NT], BF16, kind="ExternalOutput")
        dbg_vones = nc.dram_tensor(
            "dbg_vones", [128, B, JC, DH + 1], BF16, kind="ExternalOutput"
        )
        dbg_p = nc.dram_tensor("dbg_p", [128, T], BF16, kind="ExternalOutput")
        dbg_yacc = nc.dram_tensor("dbg_yacc", [DH + 1, T], F32, kind="ExternalOutput")
        dbg_rb = nc.dram_tensor("dbg_rb", [64, T], F32, kind="ExternalOutput")
        dbg_ysb = nc.dram_tensor("dbg_ysb", [2, 128, NT], BF16, kind="ExternalOutput")
        dbg_ag = nc.dram_tensor("dbg_ag", [KC, 128, NT], BF16, kind="ExternalOutput")

    with tile.TileContext(nc) as tc, ExitStack() as top:
        wpool = top.enter_context(tc.tile_pool(name="weights", bufs=1))
        wq_sb = wpool.tile([128, KC, DQ], BF16)
        wkv_sb = wpool.tile([128, KC, 128], BF16)
        nc.gpsimd.dma_start(wq_sb[:], wq[:, :, :].rearrange("k p d -> p k d"))
        nc.gpsimd.dma_start(wkv_sb[:], wkv[:, :, :].rearrange("k p d -> p k d"))

        qkv_pool = top.enter_context(tc.tile_pool(name="qkv", bufs=1))
        qT = [qkv_pool.tile([64, NT], BF16, name=f"qT{h}") for h in range(QH)]
        kT = qkv_pool.tile([64, NT], BF16, name="kT")
        vT = qkv_pool.tile([64, NT], BF16, name="vT")
        vones = qkv_pool.tile([128, B, JC, DH + 1], BF16, name="vones")
        yT_sb = [qkv_pool.tile([128, NT], BF16, name=f"yTsb{i}") for i in range(2)]
        ident_sb = wpool.tile([128, 128], BF16, name="ident_sb")
        ones_sb = wpool.tile([1, 64], F32, name="ones_sb")
        nc.gpsimd.dma_start(ident_sb[:], ident[:])
        nc.vector.memset(ones_sb[:], 1.0)

        nc.vector.memset(vones[:, :, :, DH : DH + 1], 1.0)

        # ---------------- phase 1: QKV projections (transposed layout) ------
        with tc.tile_pool(name="xcol", bufs=2) as xpool, tc.tile_pool(
            name="qkvps", bufs=3, space="PSUM"
        ) as qkvps:
            for n in range(B * TC):
                b, nn = divmod(n, TC)
                col = slice(n * 512, (n + 1) * 512)
                xc = xpool.tile([128, KC, 512], BF16, name="xc")
                nc.gpsimd.dma_start(
                    xc[:],
                    xT[b, :, :, nn * 512 : (nn + 1) * 512].rearrange(
                        "k p t -> p k t"
                    ),
                )
                for m in range(2):  # q-head pairs (2m, 2m+1)
                    ps = qkvps.tile([128, 512], F32, name="ps")
                    for k in range(KC):
                        nc.tensor.matmul(
                            ps[:],
                            wq_sb[:, k, m * 128 : (m + 1) * 128],
                            xc[:, k, :],
                            start=(k == 0),
                            stop=(k == KC - 1),
                        )
                    nc.vector.tensor_copy(qT[2 * m][0:64, col], ps[0:64, :])
                    nc.vector.tensor_copy(qT[2 * m + 1][0:64, col], ps[64:128, :])
                ps = qkvps.tile([128, 512], F32, name="ps")
                for k in range(KC):
                    nc.tensor.matmul(
                        ps[:],
                        wkv_sb[:, k, :],
                        xc[:, k, :],
                        start=(k == 0),
                        stop=(k == KC - 1),
                    )
                nc.vector.tensor_copy(kT[0:64, col], ps[0:64, :])
                nc.vector.tensor_copy(vT[0:64, col], ps[64:128, :])

        # ---------------- phase 1.5: V to natural layout (PE transpose) -----
        with tc.tile_pool(name="tps", bufs=3, space="PSUM") as tpool:
            for b in range(B):
                for j in range(JC):
                    tp = tpool.tile([128, DH], BF16, name="tp")
                    nc.tensor.transpose(
                        tp[:],
                        vT[0:64, b * T + j * 128 : b * T + (j + 1) * 128],
                        ident_sb[0:64, 0:64],
                    )
                    nc.vector.tensor_copy(vones[:, b, j, 0:DH], tp[:])
        if debug_dumps:
            nc.gpsimd.dma_start(dbg_q[:], qT[0][:])
            nc.gpsimd.dma_start(dbg_k[:], kT[:])
            nc.gpsimd.dma_start(dbg_vones[:], vones[:])

        # DRAM bounce buffers for the per-batch AllGathers (issued inside the
        # attention loop so AG(b0) overlaps batch-1 attention compute).
        dpool = top.enter_context(tc.tile_pool(name="dram", bufs=1, space="DRAM"))
        yT_in_b = [
            dpool.tile([DQ, T], BF16, name=f"yTin{b}") for b in range(B)
        ]
        yT_all_b = [
            dpool.tile([KC, 128, T], BF16, addr_space="Shared", name=f"yTall{b}")
            for b in range(B)
        ]

        # ---------------- phase 2: attention ------------------------------
        with tc.tile_pool(name="spool", bufs=1, space="PSUM") as spool, tc.tile_pool(
            name="ypool", bufs=1, space="PSUM"
        ) as ypsum, tc.tile_pool(name="ppool", bufs=3) as ppool, tc.tile_pool(
            name="mpool", bufs=1
        ) as mpool, tc.tile_pool(name="rpool", bufs=2) as rpool:
            mask_sb = mpool.tile([128, JC, mw], BF16, name="mask_sb")
            nc.gpsimd.dma_start(
                mask_sb[:], maskT[:, :, :].rearrange("j p w -> p j w")
            )
            for b in range(B):
                for h in range(QH):
                    y_acc = ypsum.tile([DH + 1, T], F32, name="y_acc")
                    for j in range(JC):
                        q0 = 512 * (j // 4) if causal else 0
                        w = T - q0
                        S = spool.tile([128, T], F32, name="S")
                        lk = kT[0:64, b * T + j * 128 : b * T + (j + 1) * 128]
                        for u in range(w // 512):
                            nc.tensor.matmul(
                                S[:, u * 512 : (u + 1) * 512],
                                lk,
                                qT[h][
                                    0:64,
                                    b * T + q0 + u * 512 : b * T + q0 + (u + 1) * 512,
                                ],
                                start=True,
                                stop=True,
                            )
                        nc.vector.tensor_add(
                            S[:, 0:mw], S[:, 0:mw], mask_sb[:, j, :]
                        )
                        pT = ppool.tile([128, T], BF16, name="pT")
                        nc.scalar.activation(pT[:, 0:w], S[:, 0:w], EXP)
                        if debug_dumps and b == 0 and h == 0 and j == 0:
                            nc.gpsimd.dma_start(dbg_p[:], pT[:])
                        for n2 in range(q0 // 512, TC):
                            last_j = 4 * n2 + 3 if causal else JC - 1
                            nc.tensor.matmul(
                                y_acc[:, n2 * 512 : (n2 + 1) * 512],
                                vones[:, b, j, :],
                                pT[:, n2 * 512 - q0 : n2 * 512 - q0 + 512],
                                start=(j == 0),
                                stop=(j == last_j),
                            )
                    r_sb = rpool.tile([1, T], F32, name="r_sb")
                    nc.vector.reciprocal(r_sb[:], y_acc[DH : DH + 1, :])
                    rb_ps = spool.tile([64, T], F32, name="rb_ps", tag="S")
                    for u in range(TC):
                        nc.tensor.matmul(
                            rb_ps[:, u * 512 : (u + 1) * 512],
                            ones_sb[:],
                            r_sb[:, u * 512 : (u + 1) * 512],
                            start=True,
                            stop=True,
                        )
                    rb = rpool.tile([64, T], F32, name="rb")
                    nc.vector.tensor_copy(rb[:], rb_ps[:])
                    if debug_dumps and b == 0 and h == 0:
                        yacc_sb = rpool.tile([DH + 1, T], F32, name="yacc_sb")
                        nc.vector.tensor_copy(yacc_sb[:], y_acc[:])
                        nc.gpsimd.dma_start(dbg_yacc[:], yacc_sb[:])
                        nc.gpsimd.dma_start(dbg_rb[:], rb[:])
                    dst = yT_sb[h // 2][
                        64 * (h % 2) : 64 * (h % 2) + 64, b * T : (b + 1) * T
                    ]
                    nc.vector.tensor_mul(dst, y_acc[0:DH, :], rb[:])
                # AllGather this batch's y^T block while the next batch's
                # attention runs. (SWDGE DMAs only near collectives — see
                # test_sync_dma_collective_hang.)
                nc.gpsimd.dma_start(
                    yT_in_b[b][0:128, :], yT_sb[0][:, b * T : (b + 1) * T]
                )
                nc.gpsimd.dma_start(
                    yT_in_b[b][128:256, :], yT_sb[1][:, b * T : (b + 1) * T]
                )
                nc.gpsimd.collective_compute(
                    "AllGather",
                    mybir.AluOpType.bypass,
                    replica_groups=[list(range(NCORES))],
                    ins=[yT_in_b[b].opt()],
                    outs=[yT_all_b[b].opt()],
                )

        if debug_dumps:
            nc.gpsimd.dma_start(dbg_ysb[0], yT_sb[0][:])
            nc.gpsimd.dma_start(dbg_ysb[1], yT_sb[1][:])
            for b in range(B):
                nc.gpsimd.dma_start(
                    dbg_ag[:, :, b * T : (b + 1) * T], yT_all_b[b][:]
                )

        if True:
            # ------------- phase 4: output projection (out^T shard) --------
            with tc.tile_pool(name="ysl", bufs=3) as ylp, tc.tile_pool(
                name="wopool", bufs=1
            ) as wop, tc.tile_pool(
                name="popool", bufs=8, space="PSUM"
            ) as pop, tc.tile_pool(name="osb", bufs=2) as osp:
                wo_sb = wop.tile([128, KC, DQ], BF16, name="wo_sb")
                for k in range(KC):
                    nc.gpsimd.dma_start(wo_sb[:, k, :], wo[k])
                for half in range(2):
                    pos = [
                        [pop.tile([128, 512], F32, name="po") for _ in range(4)]
                        for _ in range(2)
                    ]
                    for k in range(KC):
                        ysl = ylp.tile([128, 2048], BF16, name="ysl")
                        nc.gpsimd.dma_start(ysl[:], yT_all_b[half][k, :, :])
                        for m in range(2):
                            for u in range(4):
                                nc.tensor.matmul(
                                    pos[m][u][:],
                                    wo_sb[:, k, m * 128 : (m + 1) * 128],
                                    ysl[:, u * 512 : (u + 1) * 512],
                                    start=(k == 0),
                                    stop=(k == KC - 1),
                                )
                    for m in range(2):
                        for u in range(4):
                            osb = osp.tile([128, 512], F32, name="osb")
                            nc.vector.tensor_copy(osb[:], pos[m][u][:])
                            nc.gpsimd.dma_start(
                                outT[
                                    m,
                                    :,
                                    half * 2048 + u * 512 : half * 2048 + (u + 1) * 512,
                                ],
                                osb[:],
                            )
    if compile:
        nc.compile()
    return nc


_CACHE: dict = {}


def _get_compiled(causal: bool) -> bacc.Bacc:
    if causal not in _CACHE:
        _CACHE[causal] = _build(causal)
    return _CACHE[causal]


def _prep_inputs(x, attn_mask, Wq, Wk, Wv, Wo, causal):
    x = np.asarray(x, dtype=np.float32)
    mask2d = np.asarray(attn_mask, dtype=np.float32).reshape(T, T)
    Wq = np.asarray(Wq, dtype=np.float32) * 0.125  # fold 1/sqrt(64) into Wq
    Wk = np.asarray(Wk, dtype=np.float32)
    Wv = np.asarray(Wv, dtype=np.float32)
    Wo = np.asarray(Wo, dtype=np.float32)

    xT = (
        np.ascontiguousarray(x.transpose(0, 2, 1))
        .reshape(B, KC, 128, T)
        .astype(NPBF16)
    )
    if causal:
        maskT = np.stack(
            [
                mask2d[
                    512 * (j // 4) : 512 * (j // 4) + 512, 128 * j : 128 * (j + 1)
                ].T
                for j in range(JC)
            ]
        ).astype(NPBF16)
    else:
        maskT = np.stack(
            [mask2d[:, 128 * j : 128 * (j + 1)].T for j in range(JC)]
        ).astype(NPBF16)

    in_maps = []
    for c in range(NCORES):
        wq_c = np.ascontiguousarray(Wq[:, c * DQ : (c + 1) * DQ]).reshape(
            KC, 128, DQ
        ).astype(NPBF16)
        wkv_c = np.concatenate(
            [Wk[:, c * DH : (c + 1) * DH], Wv[:, c * DH : (c + 1) * DH]], axis=1
        ).reshape(KC, 128, 128).astype(NPBF16)
        wo_c = np.ascontiguousarray(Wo[:, c * DQ : (c + 1) * DQ]).reshape(
            KC, 128, DQ
        ).astype(NPBF16)
        in_maps.append(
            {
                "xT": xT,
                "wq": wq_c,
                "wkv": wkv_c,
                "wo": wo_c,
                "maskT": maskT,
                "ident": np.eye(128, dtype=NPBF16),
            }
        )
    return in_maps


def _is_causal(attn_mask) -> bool:
    mask2d = np.asarray(attn_mask, dtype=np.float32).reshape(T, T)
    ref = np.triu(np.full((T, T), -1e9, dtype=np.float32), k=1)
    return bool(np.array_equal(mask2d, ref))


def _run(x, attn_mask, Wq, Wk, Wv, Wo, trace=False, trace_cores=None):
    causal = _is_causal(attn_mask)
    nc = _get_compiled(causal)
    in_maps = _prep_inputs(x, attn_mask, Wq, Wk, Wv, Wo, causal)
    res = run_bass_kernel_spmd(
        nc,
        in_maps,
        core_ids=list(range(NCORES)),
        trace=trace,
        trace_cores=trace_cores,
    )
    outT = np.concatenate(
        [np.asarray(r["outT"], dtype=np.float32).reshape(DQ, NT) for r in res.results],
        axis=0,
    )
    out = np.ascontiguousarray(outT.T).reshape(B, T, HID).astype(np.float32)
    return out, res


def kernel(x, attn_mask, Wq, Wk, Wv, Wo):
    out, _ = _run(x, attn_mask, Wq, Wk, Wv, Wo)
    return out


if __name__ == "__main__":
    rng = np.random.default_rng(0)
    x = rng.standard_normal((B, T, HID), dtype=np.float32)
    mask = np.triu(np.full((T, T), -1e9, dtype=np.float32), k=1)[None, None]
    s = 1.0 / np.sqrt(HID)
    Wq = rng.standard_normal((HID, HEADS * DH), dtype=np.float32) * s
    Wk = rng.standard_normal((HID, KV_HEADS * DH), dtype=np.float32) * s
    Wv = rng.standard_normal((HID, KV_HEADS * DH), dtype=np.float32) * s
    Wo = rng.standard_normal((HEADS * DH, HID), dtype=np.float32) * s
    out = kernel(x, mask, Wq, Wk, Wv, Wo)
    print("out", out.shape, out.dtype, np.abs(out).mean())



# revision 2
# speedup vs baseline: 1.6788x; 1.6788x over previous
"""GQA attention (B=2, T=2048, HID=2048, 32 q-heads / 8 kv-heads, d=64)
distributed over 8 TRN2 NeuronCores.

Sharding: tensor-parallel over heads. Core c owns q-heads [4c, 4c+4) and
kv-head c (column shards of Wq/Wk/Wv), plus the matching column shard of Wo
used to compute out^T rows. x is replicated (host pre-transposes to [hid, tok]
and casts to bf16). After local attention each core AllGathers its y^T
[256, 4096] block into the full y^T [2048, 4096], then computes
out^T[256c:256c+256, :] locally. The host concatenates and transposes.

Fast causal path design notes:
- PE p-states: the PE only reaches 2.4 GHz after ~3us of continuous busy.
  The attention j-loop is software-pipelined (scores run 2 iterations ahead
  of PV) with triple-buffered S/pT tiles so the PE never stalls on the
  exp (ACT engine) round-trip.
- Softmax denominator: V is augmented with 64 columns of ones, so the PV
  matmul itself produces the denominator replicated across 64 PSUM
  partitions (matmul cost depends only on N, not M). Normalization is then
  one DVE reciprocal [64,512] + one DVE multiply per (head, q-block).
- Causal trim: scores/exp are only computed for the live region at
  128-column granularity; diagonal-block pT tiles keep a pre-zeroed
  masked prefix so PV can run untrimmed with correct start/stop flags.
- Heads processed in sweeps of 2 so PSUM fits: S pair tiles [128,2,512]
  x3 bufs (6 banks) + 2 y_acc [128,512] (2 banks) = 8 banks.

All matmuls run in bf16 with f32 PSUM accumulation. Softmax runs without
max-subtraction (scores are O(10) for this distribution; exp is exact in
f32) and exp(-1e9) underflows to exactly 0 for the masked region.
"""

import os
import sys

import numpy as np

for _p in ("/opt/trn_rl_repo", "/root/.axon_site/_ro/trn_rl_repo"):
    if os.path.isdir(_p) and _p not in sys.path:
        sys.path.append(_p)

import ml_dtypes  # noqa: E402
from contextlib import ExitStack  # noqa: E402

import concourse.bass as bass  # noqa: E402
import concourse.tile as tile  # noqa: E402
from concourse import bacc, mybir  # noqa: E402
from concourse.bass_utils import run_bass_kernel_spmd  # noqa: E402

BF16 = mybir.dt.bfloat16
F32 = mybir.dt.float32
NPBF16 = ml_dtypes.bfloat16

B, T, HID = 2, 2048, 2048
NT = B * T
HEADS, KV_HEADS, DH = 32, 8, 64
NCORES = 8
QH = HEADS // NCORES          # q-heads per core
DQ = QH * DH                  # 256
KC = HID // 128               # 16 hidden-dim chunks
TC = T // 512                 # 4 token chunks of 512 per batch
JC = T // 128                 # 16 key chunks of 128 per batch
EXP = mybir.ActivationFunctionType.Exp


def _build_fast(debug: bool = False, compile: bool = True) -> bacc.Bacc:
    """Fast builder for the causal mask case."""
    nc = bacc.Bacc(
        "TRN2", target_bir_lowering=False, debug=debug, num_devices=NCORES
    )
    xT = nc.dram_tensor("xT", [B, KC, 128, T], BF16, kind="ExternalInput")
    wq = nc.dram_tensor("wq", [KC, 128, DQ], BF16, kind="ExternalInput")
    wkv = nc.dram_tensor("wkv", [KC, 128, 128], BF16, kind="ExternalInput")
    wo = nc.dram_tensor("wo", [KC, 128, DQ], BF16, kind="ExternalInput")
    tri = nc.dram_tensor("tri", [128, 128], BF16, kind="ExternalInput")
    ident = nc.dram_tensor("ident", [128, 128], BF16, kind="ExternalInput")
    outT = nc.dram_tensor("outT", [2, 128, NT], F32, kind="ExternalOutput")

    with tile.TileContext(nc) as tc, ExitStack() as top:
        wpool = top.enter_context(tc.tile_pool(name="weights", bufs=1))
        wq_sb = wpool.tile([128, KC, DQ], BF16)
        wkv_sb = wpool.tile([128, KC, 128], BF16)
        wo_sb = wpool.tile([128, KC, DQ], BF16)
        ident_sb = wpool.tile([128, 128], BF16, name="ident_sb")
        tri_sb = wpool.tile([128, 128], BF16, name="tri_sb")
        nc.gpsimd.dma_start(wq_sb[:], wq[:, :, :].rearrange("k p d -> p k d"))
        nc.gpsimd.dma_start(wkv_sb[:], wkv[:, :, :].rearrange("k p d -> p k d"))
        nc.gpsimd.dma_start(wo_sb[:], wo[:, :, :].rearrange("k p d -> p k d"))
        nc.gpsimd.dma_start(ident_sb[:], ident[:])
        nc.gpsimd.dma_start(tri_sb[:], tri[:])

        qkv_pool = top.enter_context(tc.tile_pool(name="qkv", bufs=1))
        qT = [qkv_pool.tile([64, NT], BF16, name=f"qT{h}") for h in range(QH)]
        kT = qkv_pool.tile([64, NT], BF16, name="kT")
        vT = qkv_pool.tile([64, NT], BF16, name="vT")
        # V augmented with 64 ones-columns: PV matmul then emits the softmax
        # denominator replicated on PSUM partitions 64..127.
        vones = qkv_pool.tile([128, B, JC, 128], BF16, name="vones")
        yT_sb = [qkv_pool.tile([128, NT], BF16, name=f"yTsb{i}") for i in range(2)]
        nc.vector.memset(vones[:, :, :, DH:128], 1.0)

        # pT pools: rotating tiles for off-diagonal blocks, plus one
        # dedicated tile per diagonal offset r with a permanently-zero
        # masked prefix (exp never writes columns < 128r).
        pt_pool = top.enter_context(tc.tile_pool(name="ptf", bufs=3))
        ptd_pool = top.enter_context(tc.tile_pool(name="ptd", bufs=1))
        ptd = [
            ptd_pool.tile([128, 2, 512], BF16, name=f"ptd{r}") for r in range(4)
        ]
        for r in range(1, 4):
            nc.vector.memset(ptd[r][:, :, 0 : 128 * r], 0.0)

        # DRAM bounce buffers for the per-batch AllGathers (issued inside the
        # batch loop so AG(b0) overlaps batch-1 compute). SWDGE DMAs only
        # near collectives — see test_sync_dma_collective_hang.
        dpool = top.enter_context(tc.tile_pool(name="dram", bufs=1, space="DRAM"))
        yT_in_b = [dpool.tile([DQ, T], BF16, name=f"yTin{b}") for b in range(B)]
        yT_all_b = [
            dpool.tile([KC, 128, T], BF16, addr_space="Shared", name=f"yTall{b}")
            for b in range(B)
        ]

        for b in range(B):
            # ---------------- QKV projections for batch b -------------------
            with tc.tile_pool(name="xcol", bufs=2) as xpool, tc.tile_pool(
                name="qkvps", bufs=3, space="PSUM"
            ) as qkvps, tc.tile_pool(name="tps", bufs=2, space="PSUM") as tpool:
                for nn in range(TC):
                    col = slice(b * T + nn * 512, b * T + (nn + 1) * 512)
                    xc = xpool.tile([128, KC, 512], BF16, name="xc")
                    nc.gpsimd.dma_start(
                        xc[:],
                        xT[b, :, :, nn * 512 : (nn + 1) * 512].rearrange(
                            "k p t -> p k t"
                        ),
                    )
                    for m in range(2):  # q-head pairs (2m, 2m+1)
                        ps = qkvps.tile([128, 512], F32, name="ps")
                        for k in range(KC):
                            nc.tensor.matmul(
                                ps[:],
                                wq_sb[:, k, m * 128 : (m + 1) * 128],
                                xc[:, k, :],
                                start=(k == 0),
                                stop=(k == KC - 1),
                            )
                        nc.vector.tensor_copy(qT[2 * m][0:64, col], ps[0:64, :])
                        nc.vector.tensor_copy(qT[2 * m + 1][0:64, col], ps[64:128, :])
                    ps = qkvps.tile([128, 512], F32, name="ps")
                    for k in range(KC):
                        nc.tensor.matmul(
                            ps[:],
                            wkv_sb[:, k, :],
                            xc[:, k, :],
                            start=(k == 0),
                            stop=(k == KC - 1),
                        )
                    nc.vector.tensor_copy(kT[0:64, col], ps[0:64, :])
                    nc.vector.tensor_copy(vT[0:64, col], ps[64:128, :])

                # V to natural layout (PE transpose) for batch b
                for j in range(JC):
                    tp = tpool.tile([128, DH], BF16, name="tp")
                    nc.tensor.transpose(
                        tp[:],
                        vT[0:64, b * T + j * 128 : b * T + (j + 1) * 128],
                        ident_sb[0:64, 0:64],
                    )
                    nc.vector.tensor_copy(vones[:, b, j, 0:DH], tp[:])

            # ---------------- attention for batch b -------------------------
            with tc.tile_pool(name="spool", bufs=3, space="PSUM") as spool, tc.tile_pool(
                name="ypool", bufs=1, space="PSUM"
            ) as ypool, tc.tile_pool(name="rpool", bufs=2) as rpool:
                for hg in range(2):  # head pairs (2hg, 2hg+1)
                    for n2 in range(TC):
                        y_acc = [
                            ypool.tile([128, 512], F32, name=f"ya{hh}")
                            for hh in range(2)
                        ]
                        nj = 4 * (n2 + 1)
                        qb = b * T + n2 * 512
                        pend = []  # software pipeline: PV lags scores by 2

                        def emit_pv(ent):
                            pj, ppt = ent
                            for hh in range(2):
                                nc.tensor.matmul(
                                    y_acc[hh][:, :],
                                    vones[:, b, pj, :],
                                    ppt[:, hh, :],
                                    start=(pj == 0),
                                    stop=(pj == nj - 1),
                                )

                        for j in range(nj):
                            diag = j >= 4 * n2
                            off = 128 * (j - 4 * n2) if diag else 0
                            S = spool.tile([128, 2, 512], F32, name="S")
                            lk = kT[0:64, b * T + j * 128 : b * T + (j + 1) * 128]
                            for hh in range(2):
                                nc.tensor.matmul(
                                    S[:, hh, off:512],
                                    lk,
                                    qT[2 * hg + hh][0:64, qb + off : qb + 512],
                                    start=True,
                                    stop=True,
                                )
                            if diag:
                                for hh in range(2):
                                    nc.vector.tensor_add(
                                        S[:, hh, off : off + 128],
                                        S[:, hh, off : off + 128],
                                        tri_sb[:],
                                    )
                                pt = ptd[j - 4 * n2]
                            else:
                                pt = pt_pool.tile([128, 2, 512], BF16, name="pt")
                            nc.scalar.activation(
                                pt[:, :, off:512], S[:, :, off:512], EXP
                            )
                            pend.append((j, pt))
                            if len(pend) > 2:
                                emit_pv(pend.pop(0))
                        for ent in pend:
                            emit_pv(ent)

                        # normalize + store y^T rows for this q-block
                        for hh in range(2):
                            rec = rpool.tile([64, 512], F32, name="rec")
                            nc.vector.reciprocal(rec[:], y_acc[hh][64:128, :])
                            nc.vector.tensor_mul(
                                yT_sb[hg][
                                    64 * hh : 64 * hh + 64, qb : qb + 512
                                ],
                                y_acc[hh][0:64, :],
                                rec[:],
                            )

            # ---------------- AllGather this batch's y^T block --------------
            nc.gpsimd.dma_start(
                yT_in_b[b][0:128, :], yT_sb[0][:, b * T : (b + 1) * T]
            )
            nc.gpsimd.dma_start(
                yT_in_b[b][128:256, :], yT_sb[1][:, b * T : (b + 1) * T]
            )
            nc.gpsimd.collective_compute(
                "AllGather",
                mybir.AluOpType.bypass,
                replica_groups=[list(range(NCORES))],
                ins=[yT_in_b[b].opt()],
                outs=[yT_all_b[b].opt()],
            )

        # ---------------- output projection (out^T shard) -------------------
        with tc.tile_pool(name="ysl", bufs=3) as ylp, tc.tile_pool(
            name="popool", bufs=8, space="PSUM"
        ) as pop, tc.tile_pool(name="osb", bufs=2) as osp:
            for half in range(2):
                pos = [
                    [pop.tile([128, 512], F32, name="po") for _ in range(4)]
                    for _ in range(2)
                ]
                for k in range(KC):
                    ysl = ylp.tile([128, 2048], BF16, name="ysl")
                    nc.gpsimd.dma_start(ysl[:], yT_all_b[half][k, :, :])
                    for m in range(2):
                        for u in range(4):
                            nc.tensor.matmul(
                                pos[m][u][:],
                                wo_sb[:, k, m * 128 : (m + 1) * 128],
                                ysl[:, u * 512 : (u + 1) * 512],
                                start=(k == 0),
                                stop=(k == KC - 1),
                            )
                for m in range(2):
                    for u in range(4):
                        osb = osp.tile([128, 512], F32, name="osb")
                        nc.vector.tensor_copy(osb[:], pos[m][u][:])
                        nc.gpsimd.dma_start(
                            outT[
                                m,
                                :,
                                half * 2048 + u * 512 : half * 2048 + (u + 1) * 512,
                            ],
                            osb[:],
                        )
    if compile:
        nc.compile()
    return nc


def _build_ref(causal: bool, debug: bool = False, compile: bool = True) -> bacc.Bacc:
    """Reference builder (handles arbitrary masks; used for non-causal)."""
    nc = bacc.Bacc(
        "TRN2", target_bir_lowering=False, debug=debug, num_devices=NCORES
    )
    xT = nc.dram_tensor("xT", [B, KC, 128, T], BF16, kind="ExternalInput")
    wq = nc.dram_tensor("wq", [KC, 128, DQ], BF16, kind="ExternalInput")
    wkv = nc.dram_tensor("wkv", [KC, 128, 128], BF16, kind="ExternalInput")
    wo = nc.dram_tensor("wo", [KC, 128, DQ], BF16, kind="ExternalInput")
    mw = 512 if causal else T
    maskT = nc.dram_tensor("maskT", [JC, 128, mw], BF16, kind="ExternalInput")
    ident = nc.dram_tensor("ident", [128, 128], BF16, kind="ExternalInput")
    outT = nc.dram_tensor("outT", [2, 128, NT], F32, kind="ExternalOutput")

    with tile.TileContext(nc) as tc, ExitStack() as top:
        wpool = top.enter_context(tc.tile_pool(name="weights", bufs=1))
        wq_sb = wpool.tile([128, KC, DQ], BF16)
        wkv_sb = wpool.tile([128, KC, 128], BF16)
        nc.gpsimd.dma_start(wq_sb[:], wq[:, :, :].rearrange("k p d -> p k d"))
        nc.gpsimd.dma_start(wkv_sb[:], wkv[:, :, :].rearrange("k p d -> p k d"))

        qkv_pool = top.enter_context(tc.tile_pool(name="qkv", bufs=1))
        qT = [qkv_pool.tile([64, NT], BF16, name=f"qT{h}") for h in range(QH)]
        kT = qkv_pool.tile([64, NT], BF16, name="kT")
        vT = qkv_pool.tile([64, NT], BF16, name="vT")
        vones = qkv_pool.tile([128, B, JC, DH + 1], BF16, name="vones")
        yT_sb = [qkv_pool.tile([128, NT], BF16, name=f"yTsb{i}") for i in range(2)]
        ident_sb = wpool.tile([128, 128], BF16, name="ident_sb")
        ones_sb = wpool.tile([1, 64], F32, name="ones_sb")
        nc.gpsimd.dma_start(ident_sb[:], ident[:])
        nc.vector.memset(ones_sb[:], 1.0)

        nc.vector.memset(vones[:, :, :, DH : DH + 1], 1.0)

        # ---------------- phase 1: QKV projections (transposed layout) ------
        with tc.tile_pool(name="xcol", bufs=2) as xpool, tc.tile_pool(
            name="qkvps", bufs=3, space="PSUM"
        ) as qkvps:
            for n in range(B * TC):
                b, nn = divmod(n, TC)
                col = slice(n * 512, (n + 1) * 512)
                xc = xpool.tile([128, KC, 512], BF16, name="xc")
                nc.gpsimd.dma_start(
                    xc[:],
                    xT[b, :, :, nn * 512 : (nn + 1) * 512].rearrange(
                        "k p t -> p k t"
                    ),
                )
                for m in range(2):  # q-head pairs (2m, 2m+1)
                    ps = qkvps.tile([128, 512], F32, name="ps")
                    for k in range(KC):
                        nc.tensor.matmul(
                            ps[:],
                            wq_sb[:, k, m * 128 : (m + 1) * 128],
                            xc[:, k, :],
                            start=(k == 0),
                            stop=(k == KC - 1),
                        )
                    nc.vector.tensor_copy(qT[2 * m][0:64, col], ps[0:64, :])
                    nc.vector.tensor_copy(qT[2 * m + 1][0:64, col], ps[64:128, :])
                ps = qkvps.tile([128, 512], F32, name="ps")
                for k in range(KC):
                    nc.tensor.matmul(
                        ps[:],
                        wkv_sb[:, k, :],
                        xc[:, k, :],
                        start=(k == 0),
                        stop=(k == KC - 1),
                    )
                nc.vector.tensor_copy(kT[0:64, col], ps[0:64, :])
                nc.vector.tensor_copy(vT[0:64, col], ps[64:128, :])

        # ---------------- phase 1.5: V to natural layout (PE transpose) -----
        with tc.tile_pool(name="tps", bufs=3, space="PSUM") as tpool:
            for b in range(B):
                for j in range(JC):
                    tp = tpool.tile([128, DH], BF16, name="tp")
                    nc.tensor.transpose(
                        tp[:],
                        vT[0:64, b * T + j * 128 : b * T + (j + 1) * 128],
                        ident_sb[0:64, 0:64],
                    )
                    nc.vector.tensor_copy(vones[:, b, j, 0:DH], tp[:])

        dpool = top.enter_context(tc.tile_pool(name="dram", bufs=1, space="DRAM"))
        yT_in_b = [dpool.tile([DQ, T], BF16, name=f"yTin{b}") for b in range(B)]
        yT_all_b = [
            dpool.tile([KC, 128, T], BF16, addr_space="Shared", name=f"yTall{b}")
            for b in range(B)
        ]

        # ---------------- phase 2: attention ------------------------------
        with tc.tile_pool(name="spool", bufs=1, space="PSUM") as spool, tc.tile_pool(
            name="ypool", bufs=1, space="PSUM"
        ) as ypsum, tc.tile_pool(name="ppool", bufs=3) as ppool, tc.tile_pool(
            name="mpool", bufs=1
        ) as mpool, tc.tile_pool(name="rpool", bufs=2) as rpool:
            mask_sb = mpool.tile([128, JC, mw], BF16, name="mask_sb")
            nc.gpsimd.dma_start(
                mask_sb[:], maskT[:, :, :].rearrange("j p w -> p j w")
            )
            for b in range(B):
                for h in range(QH):
                    y_acc = ypsum.tile([DH + 1, T], F32, name="y_acc")
                    for j in range(JC):
                        q0 = 512 * (j // 4) if causal else 0
                        w = T - q0
                        S = spool.tile([128, T], F32, name="S")
                        lk = kT[0:64, b * T + j * 128 : b * T + (j + 1) * 128]
                        for u in range(w // 512):
                            nc.tensor.matmul(
                                S[:, u * 512 : (u + 1) * 512],
                                lk,
                                qT[h][
                                    0:64,
                                    b * T + q0 + u * 512 : b * T + q0 + (u + 1) * 512,
                                ],
                                start=True,
                                stop=True,
                            )
                        nc.vector.tensor_add(
                            S[:, 0:mw], S[:, 0:mw], mask_sb[:, j, :]
                        )
                        pT = ppool.tile([128, T], BF16, name="pT")
                        nc.scalar.activation(pT[:, 0:w], S[:, 0:w], EXP)
                        for n2 in range(q0 // 512, TC):
                            last_j = 4 * n2 + 3 if causal else JC - 1
                            nc.tensor.matmul(
                                y_acc[:, n2 * 512 : (n2 + 1) * 512],
                                vones[:, b, j, :],
                                pT[:, n2 * 512 - q0 : n2 * 512 - q0 + 512],
                                start=(j == 0),
                                stop=(j == last_j),
                            )
                    r_sb = rpool.tile([1, T], F32, name="r_sb")
                    nc.vector.reciprocal(r_sb[:], y_acc[DH : DH + 1, :])
                    rb_ps = spool.tile([64, T], F32, name="rb_ps", tag="S")
                    for u in range(TC):
                        nc.tensor.matmul(
                            rb_ps[:, u * 512 : (u + 1) * 512],
                            ones_sb[:],
                            r_sb[:, u * 512 : (u + 1) * 512],
                            start=True,
                            stop=True,
                        )
                    rb = rpool.tile([64, T], F32, name="rb")
                    nc.vector.tensor_copy(rb[:], rb_ps[:])
                    dst = yT_sb[h // 2][
                        64 * (h % 2) : 64 * (h % 2) + 64, b * T : (b + 1) * T
                    ]
                    nc.vector.tensor_mul(dst, y_acc[0:DH, :], rb[:])
                nc.gpsimd.dma_start(
                    yT_in_b[b][0:128, :], yT_sb[0][:, b * T : (b + 1) * T]
                )
                nc.gpsimd.dma_start(
                    yT_in_b[b][128:256, :], yT_sb[1][:, b * T : (b + 1) * T]
                )
                nc.gpsimd.collective_compute(
                    "AllGather",
                    mybir.AluOpType.bypass,
                    replica_groups=[list(range(NCORES))],
                    ins=[yT_in_b[b].opt()],
                    outs=[yT_all_b[b].opt()],
                )

        # ------------- phase 4: output projection (out^T shard) --------
        with tc.tile_pool(name="ysl", bufs=3) as ylp, tc.tile_pool(
            name="wopool", bufs=1
        ) as wop, tc.tile_pool(
            name="popool", bufs=8, space="PSUM"
        ) as pop, tc.tile_pool(name="osb", bufs=2) as osp:
            wo_sb = wop.tile([128, KC, DQ], BF16, name="wo_sb")
            for k in range(KC):
                nc.gpsimd.dma_start(wo_sb[:, k, :], wo[k])
            for half in range(2):
                pos = [
                    [pop.tile([128, 512], F32, name="po") for _ in range(4)]
                    for _ in range(2)
                ]
                for k in range(KC):
                    ysl = ylp.tile([128, 2048], BF16, name="ysl")
                    nc.gpsimd.dma_start(ysl[:], yT_all_b[half][k, :, :])
                    for m in range(2):
                        for u in range(4):
                            nc.tensor.matmul(
                                pos[m][u][:],
                                wo_sb[:, k, m * 128 : (m + 1) * 128],
                                ysl[:, u * 512 : (u + 1) * 512],
                                start=(k == 0),
                                stop=(k == KC - 1),
                            )
                for m in range(2):
                    for u in range(4):
                        osb = osp.tile([128, 512], F32, name="osb")
                        nc.vector.tensor_copy(osb[:], pos[m][u][:])
                        nc.gpsimd.dma_start(
                            outT[
                                m,
                                :,
                                half * 2048 + u * 512 : half * 2048 + (u + 1) * 512,
                            ],
                            osb[:],
                        )
    if compile:
        nc.compile()
    return nc


_CACHE: dict = {}


def _get_compiled(causal: bool) -> bacc.Bacc:
    if causal not in _CACHE:
        _CACHE[causal] = _build_fast() if causal else _build_ref(causal)
    return _CACHE[causal]


def _prep_common(x, Wq, Wk, Wv, Wo):
    x = np.asarray(x, dtype=np.float32)
    Wq = np.asarray(Wq, dtype=np.float32) * 0.125  # fold 1/sqrt(64) into Wq
    Wk = np.asarray(Wk, dtype=np.float32)
    Wv = np.asarray(Wv, dtype=np.float32)
    Wo = np.asarray(Wo, dtype=np.float32)

    xT = (
        np.ascontiguousarray(x.transpose(0, 2, 1))
        .reshape(B, KC, 128, T)
        .astype(NPBF16)
    )
    per_core = []
    for c in range(NCORES):
        wq_c = np.ascontiguousarray(Wq[:, c * DQ : (c + 1) * DQ]).reshape(
            KC, 128, DQ
        ).astype(NPBF16)
        wkv_c = np.concatenate(
            [Wk[:, c * DH : (c + 1) * DH], Wv[:, c * DH : (c + 1) * DH]], axis=1
        ).reshape(KC, 128, 128).astype(NPBF16)
        wo_c = np.ascontiguousarray(Wo[:, c * DQ : (c + 1) * DQ]).reshape(
            KC, 128, DQ
        ).astype(NPBF16)
        per_core.append((wq_c, wkv_c, wo_c))
    return xT, per_core


def _prep_inputs_fast(x, attn_mask, Wq, Wk, Wv, Wo):
    xT, per_core = _prep_common(x, Wq, Wk, Wv, Wo)
    # S[key_local, q_local] += tri[key_local, q_local]: -1e9 where key > q.
    tri = np.tril(np.full((128, 128), -1e9, dtype=np.float32), k=-1).astype(
        NPBF16
    )
    ident = np.eye(128, dtype=NPBF16)
    return [
        {
            "xT": xT,
            "wq": wq_c,
            "wkv": wkv_c,
            "wo": wo_c,
            "tri": tri,
            "ident": ident,
        }
        for (wq_c, wkv_c, wo_c) in per_core
    ]


def _prep_inputs_ref(x, attn_mask, Wq, Wk, Wv, Wo, causal):
    xT, per_core = _prep_common(x, Wq, Wk, Wv, Wo)
    mask2d = np.asarray(attn_mask, dtype=np.float32).reshape(T, T)
    if causal:
        maskT = np.stack(
            [
                mask2d[
                    512 * (j // 4) : 512 * (j // 4) + 512, 128 * j : 128 * (j + 1)
                ].T
                for j in range(JC)
            ]
        ).astype(NPBF16)
    else:
        maskT = np.stack(
            [mask2d[:, 128 * j : 128 * (j + 1)].T for j in range(JC)]
        ).astype(NPBF16)
    ident = np.eye(128, dtype=NPBF16)
    return [
        {
            "xT": xT,
            "wq": wq_c,
            "wkv": wkv_c,
            "wo": wo_c,
            "maskT": maskT,
            "ident": ident,
        }
        for (wq_c, wkv_c, wo_c) in per_core
    ]


def _is_causal(attn_mask) -> bool:
    mask2d = np.asarray(attn_mask, dtype=np.float32).reshape(T, T)
    ref = np.triu(np.full((T, T), -1e9, dtype=np.float32), k=1)
    return bool(np.array_equal(mask2d, ref))


def _run(x, attn_mask, Wq, Wk, Wv, Wo, trace=False, trace_cores=None):
    causal = _is_causal(attn_mask)
    nc = _get_compiled(causal)
    if causal:
        in_maps = _prep_inputs_fast(x, attn_mask, Wq, Wk, Wv, Wo)
    else:
        in_maps = _prep_inputs_ref(x, attn_mask, Wq, Wk, Wv, Wo, causal)
    res = run_bass_kernel_spmd(
        nc,
        in_maps,
        core_ids=list(range(NCORES)),
        trace=trace,
        trace_cores=trace_cores,
    )
    outT = np.concatenate(
        [np.asarray(r["outT"], dtype=np.float32).reshape(DQ, NT) for r in res.results],
        axis=0,
    )
    out = np.ascontiguousarray(outT.T).reshape(B, T, HID).astype(np.float32)
    return out, res


def kernel(x, attn_mask, Wq, Wk, Wv, Wo):
    out, _ = _run(x, attn_mask, Wq, Wk, Wv, Wo)
    return out


if __name__ == "__main__":
    rng = np.random.default_rng(0)
    x = rng.standard_normal((B, T, HID), dtype=np.float32)
    mask = np.triu(np.full((T, T), -1e9, dtype=np.float32), k=1)[None, None]
    s = 1.0 / np.sqrt(HID)
    Wq = rng.standard_normal((HID, HEADS * DH), dtype=np.float32) * s
    Wk = rng.standard_normal((HID, KV_HEADS * DH), dtype=np.float32) * s
    Wv = rng.standard_normal((HID, KV_HEADS * DH), dtype=np.float32) * s
    Wo = rng.standard_normal((HEADS * DH, HID), dtype=np.float32) * s
    out = kernel(x, mask, Wq, Wk, Wv, Wo)
    print("out", out.shape, out.dtype, np.abs(out).mean())


# revision 6
# speedup vs baseline: 1.7229x; 1.0263x over previous
"""GQA attention (B=2, T=2048, HID=2048, 32 q-heads / 8 kv-heads, d=64)
distributed over 8 TRN2 NeuronCores.

Sharding: tensor-parallel over heads. Core c owns q-heads [4c, 4c+4) and
kv-head c (column shards of Wq/Wk/Wv), plus the matching column shard of Wo
used to compute out^T rows. x is replicated (host pre-transposes to [hid, tok]
and casts to bf16). After local attention each core AllGathers its y^T
[256, 4096] block into the full y^T [2048, 4096], then computes
out^T[256c:256c+256, :] locally. The host concatenates and transposes.

Fast causal path design notes:
- PE p-states: the PE only reaches 2.4 GHz after ~3us of continuous busy.
  The attention j-loop is software-pipelined (scores run 2 iterations ahead
  of PV) with triple-buffered S/pT tiles so the PE never stalls on the
  exp (ACT engine) round-trip.
- Softmax denominator: V is augmented with 64 columns of ones, so the PV
  matmul itself produces the denominator replicated across 64 PSUM
  partitions (matmul cost depends only on N, not M). Normalization is then
  one DVE reciprocal [64,512] + one DVE multiply per (head, q-block).
- Causal trim: scores/exp are only computed for the live region at
  128-column granularity; diagonal-block pT tiles keep a pre-zeroed
  masked prefix so PV can run untrimmed with correct start/stop flags.
- Heads processed in sweeps of 2 so PSUM fits: S pair tiles [128,2,512]
  x3 bufs (6 banks) + 2 y_acc [128,512] (2 banks) = 8 banks.

All matmuls run in bf16 with f32 PSUM accumulation. Softmax runs without
max-subtraction (scores are O(10) for this distribution; exp is exact in
f32) and exp(-1e9) underflows to exactly 0 for the masked region.
"""

import os
import sys

import numpy as np

for _p in ("/opt/trn_rl_repo", "/root/.axon_site/_ro/trn_rl_repo"):
    if os.path.isdir(_p) and _p not in sys.path:
        sys.path.append(_p)

import ml_dtypes  # noqa: E402
from contextlib import ExitStack  # noqa: E402

import concourse.bass as bass  # noqa: E402
import concourse.tile as tile  # noqa: E402
from concourse import bacc, mybir  # noqa: E402
from concourse.bass_utils import run_bass_kernel_spmd  # noqa: E402

BF16 = mybir.dt.bfloat16
F32 = mybir.dt.float32
NPBF16 = ml_dtypes.bfloat16

B, T, HID = 2, 2048, 2048
NT = B * T
HEADS, KV_HEADS, DH = 32, 8, 64
NCORES = 8
QH = HEADS // NCORES          # q-heads per core
DQ = QH * DH                  # 256
KC = HID // 128               # 16 hidden-dim chunks
TC = T // 512                 # 4 token chunks of 512 per batch
JC = T // 128                 # 16 key chunks of 128 per batch
EXP = mybir.ActivationFunctionType.Exp


LN = mybir.ActivationFunctionType.Ln


def _build_fast(debug: bool = False, compile: bool = True) -> bacc.Bacc:
    """Fast builder for the causal mask case."""
    nc = bacc.Bacc(
        "TRN2", target_bir_lowering=False, debug=debug, num_devices=NCORES
    )
    xT = nc.dram_tensor("xT", [B, KC, 128, T], BF16, kind="ExternalInput")
    wq = nc.dram_tensor("wq", [KC, 128, DQ], BF16, kind="ExternalInput")
    wkv = nc.dram_tensor("wkv", [KC, 128, 128], BF16, kind="ExternalInput")
    wo = nc.dram_tensor("wo", [KC, 128, DQ], BF16, kind="ExternalInput")
    tri = nc.dram_tensor("tri", [128, 128], BF16, kind="ExternalInput")
    ident = nc.dram_tensor("ident", [128, 128], BF16, kind="ExternalInput")
    outT = nc.dram_tensor("outT", [2, 128, NT], F32, kind="ExternalOutput")

    with tile.TileContext(nc) as tc, ExitStack() as top:
        wpool = top.enter_context(tc.tile_pool(name="weights", bufs=1))
        wq_sb = wpool.tile([128, KC, DQ], BF16)
        wkv_sb = wpool.tile([128, KC, 128], BF16)
        wo_sb = wpool.tile([128, KC, DQ], BF16)
        ident_sb = wpool.tile([128, 128], BF16, name="ident_sb")
        tri_sb = wpool.tile([128, 128], BF16, name="tri_sb")
        nc.gpsimd.dma_start(ident_sb[:], ident[:])
        nc.gpsimd.dma_start(tri_sb[:], tri[:])
        nc.gpsimd.dma_start(wq_sb[:], wq[:, :, :].rearrange("k p d -> p k d"))
        nc.gpsimd.dma_start(wkv_sb[:], wkv[:, :, :].rearrange("k p d -> p k d"))
        nc.gpsimd.dma_start(wo_sb[:], wo[:, :, :].rearrange("k p d -> p k d"))

        qkv_pool = top.enter_context(tc.tile_pool(name="qkv", bufs=1))
        # q stored as head pairs [64, 2, NT] so one scores matmul can stream
        # both heads of a sweep (N = 2*512).
        qTp = [qkv_pool.tile([64, 2, NT], BF16, name=f"qTp{m}") for m in range(2)]
        kT = qkv_pool.tile([64, NT], BF16, name="kT")
        vT = qkv_pool.tile([64, NT], BF16, name="vT")
        # V augmented with 64 ones-columns: PV matmul then emits the softmax
        # denominator replicated on PSUM partitions 64..127.
        vones = qkv_pool.tile([128, B, JC, 128], BF16, name="vones")
        yT_sb = [qkv_pool.tile([128, NT], BF16, name=f"yTsb{i}") for i in range(2)]
        nc.vector.memset(vones[:, :, :, DH:128], 1.0)

        # pT pools: rotating tiles for off-diagonal blocks, plus one
        # dedicated tile per diagonal offset r with a permanently-zero
        # masked prefix (exp never writes columns < 128r).
        pt_pool = top.enter_context(tc.tile_pool(name="ptf", bufs=3))
        ptd_pool = top.enter_context(tc.tile_pool(name="ptd", bufs=1))
        ptd = [
            ptd_pool.tile([128, 2, 512], BF16, name=f"ptd{r}") for r in range(4)
        ]
        for r in range(1, 4):
            nc.vector.memset(ptd[r][:, :, 0 : 128 * r], 0.0)

        # DRAM bounce buffers for the per-(batch, head-pair) AllGathers
        # (issued as soon as that half of y^T is done, so all but the last
        # overlap compute). SWDGE DMAs only near collectives — see
        # test_sync_dma_collective_hang.
        dpool = top.enter_context(tc.tile_pool(name="dram", bufs=1, space="DRAM"))
        yT_in = [
            [dpool.tile([128, T], BF16, name=f"yTin{b}_{hg}") for hg in range(2)]
            for b in range(B)
        ]
        yT_all = [
            [
                dpool.tile(
                    [NCORES, 128, T],
                    BF16,
                    addr_space="Shared",
                    name=f"yTall{b}_{hg}",
                )
                for hg in range(2)
            ]
            for b in range(B)
        ]

        # ---------------- phase 1: QKV projections (both batches) -----------
        with tc.tile_pool(name="xcol", bufs=2) as xpool, tc.tile_pool(
            name="qkvps", bufs=3, space="PSUM"
        ) as qkvps, tc.tile_pool(name="tps", bufs=2, space="PSUM") as tpool:
            for n in range(NT // 512):
                col = slice(n * 512, (n + 1) * 512)
                b, nn = divmod(n, TC)
                xc = xpool.tile([128, KC, 512], BF16, name="xc")
                # split the load so the first matmuls start early
                for kq in range(4):
                    nc.gpsimd.dma_start(
                        xc[:, kq * 4 : (kq + 1) * 4, :],
                        xT[
                            b, kq * 4 : (kq + 1) * 4, :, nn * 512 : (nn + 1) * 512
                        ].rearrange("k p t -> p k t"),
                    )
                for m in range(2):  # q-head pairs (2m, 2m+1)
                    ps = qkvps.tile([128, 512], F32, name="ps")
                    for k in range(KC):
                        nc.tensor.matmul(
                            ps[:],
                            wq_sb[:, k, m * 128 : (m + 1) * 128],
                            xc[:, k, :],
                            start=(k == 0),
                            stop=(k == KC - 1),
                        )
                    nc.vector.tensor_copy(qTp[m][0:64, 0, col], ps[0:64, :])
                    nc.vector.tensor_copy(qTp[m][0:64, 1, col], ps[64:128, :])
                ps = qkvps.tile([128, 512], F32, name="ps")
                for k in range(KC):
                    nc.tensor.matmul(
                        ps[:],
                        wkv_sb[:, k, :],
                        xc[:, k, :],
                        start=(k == 0),
                        stop=(k == KC - 1),
                    )
                nc.vector.tensor_copy(kT[0:64, col], ps[0:64, :])
                nc.vector.tensor_copy(vT[0:64, col], ps[64:128, :])

            # V to natural layout (PE transpose)
            for b in range(B):
                for j in range(JC):
                    tp = tpool.tile([128, DH], BF16, name="tp")
                    nc.tensor.transpose(
                        tp[:],
                        vT[0:64, b * T + j * 128 : b * T + (j + 1) * 128],
                        ident_sb[0:64, 0:64],
                    )
                    nc.vector.tensor_copy(vones[:, b, j, 0:DH], tp[:])

        # ---------------- phase 2: attention --------------------------------
        with tc.tile_pool(name="spool", bufs=2, space="PSUM") as spool, tc.tile_pool(
            name="ypool", bufs=2, space="PSUM"
        ) as ypool, tc.tile_pool(name="rpool", bufs=2) as rpool:
            for b in range(B):
                for hg in range(2):  # head pair (2hg, 2hg+1)
                    for n2 in range(TC):
                        y_acc = ypool.tile([128, 2, 512], F32, name="ya")
                        nj = 4 * (n2 + 1)
                        qb = b * T + n2 * 512
                        pend = []  # software pipeline: PV lags scores by 2

                        def emit_pv(ent):
                            pj, ppt = ent
                            for hh in range(2):
                                nc.tensor.matmul(
                                    y_acc[:, hh, :],
                                    vones[:, b, pj, :],
                                    ppt[:, hh, :],
                                    start=(pj == 0),
                                    stop=(pj == nj - 1),
                                )

                        for j in range(nj):
                            diag = j >= 4 * n2
                            off = 128 * (j - 4 * n2) if diag else 0
                            S = spool.tile([128, 2, 512], F32, name="S")
                            lk = kT[0:64, b * T + j * 128 : b * T + (j + 1) * 128]
                            for hh in range(2):
                                nc.tensor.matmul(
                                    S[:, hh, off:512],
                                    lk,
                                    qTp[hg][0:64, hh, qb + off : qb + 512],
                                    start=True,
                                    stop=True,
                                )
                            if diag:
                                for hh in range(2):
                                    nc.vector.tensor_add(
                                        S[:, hh, off : off + 128],
                                        S[:, hh, off : off + 128],
                                        tri_sb[:],
                                    )
                                pt = ptd[j - 4 * n2]
                            else:
                                pt = pt_pool.tile([128, 2, 512], BF16, name="pt")
                            nc.scalar.activation(
                                pt[:, :, off:512], S[:, :, off:512], EXP
                            )
                            pend.append((j, pt))
                            if len(pend) > 2:
                                emit_pv(pend.pop(0))
                        for ent in pend:
                            emit_pv(ent)

                        # normalize: rec = exp(-ln(d)) on ACT, multiply on DVE
                        for hh in range(2):
                            lnd = rpool.tile([64, 512], F32, name="lnd")
                            rec = rpool.tile([64, 512], F32, name="rec")
                            nc.scalar.activation(
                                lnd[:], y_acc[64:128, hh, :], LN
                            )
                            nc.scalar.activation(rec[:], lnd[:], EXP, scale=-1.0)
                            nc.vector.tensor_mul(
                                yT_sb[hg][
                                    64 * hh : 64 * hh + 64, qb : qb + 512
                                ],
                                y_acc[0:64, hh, :],
                                rec[:],
                            )

                    # AllGather this (batch, head-pair) block of y^T
                    nc.gpsimd.dma_start(
                        yT_in[b][hg][:], yT_sb[hg][:, b * T : (b + 1) * T]
                    )
                    nc.gpsimd.collective_compute(
                        "AllGather",
                        mybir.AluOpType.bypass,
                        replica_groups=[list(range(NCORES))],
                        ins=[yT_in[b][hg].opt()],
                        outs=[yT_all[b][hg].opt()],
                    )

        # ---------------- phase 3: output projection (out^T shard) ----------
        # y^T chunk k (128 rows) comes from core k//2, head-pair k%2; consume
        # head-pair-0 chunks first so each half only waits on its first AG.
        with tc.tile_pool(name="ysl", bufs=3) as ylp, tc.tile_pool(
            name="popool", bufs=1, space="PSUM"
        ) as pop, tc.tile_pool(name="osb", bufs=2) as osp:
            korder = [2 * c for c in range(NCORES)] + [
                2 * c + 1 for c in range(NCORES)
            ]
            for half in range(2):
                pos = [
                    [pop.tile([128, 512], F32, name=f"po{m}{u}") for u in range(4)]
                    for m in range(2)
                ]
                for ki, k in enumerate(korder):
                    ysl = ylp.tile([128, 2048], BF16, name="ysl")
                    nc.gpsimd.dma_start(ysl[:], yT_all[half][k % 2][k // 2, :, :])
                    for m in range(2):
                        for u in range(4):
                            nc.tensor.matmul(
                                pos[m][u][:],
                                wo_sb[:, k, m * 128 : (m + 1) * 128],
                                ysl[:, u * 512 : (u + 1) * 512],
                                start=(ki == 0),
                                stop=(ki == KC - 1),
                            )
                for m in range(2):
                    for u in range(4):
                        osb = osp.tile([128, 512], F32, name="osb")
                        nc.vector.tensor_copy(osb[:], pos[m][u][:])
                        nc.gpsimd.dma_start(
                            outT[
                                m,
                                :,
                                half * 2048 + u * 512 : half * 2048 + (u + 1) * 512,
                            ],
                            osb[:],
                        )
    if compile:
        nc.compile()
    return nc


def _build_ref(causal: bool, debug: bool = False, compile: bool = True) -> bacc.Bacc:
    """Reference builder (handles arbitrary masks; used for non-causal)."""
    nc = bacc.Bacc(
        "TRN2", target_bir_lowering=False, debug=debug, num_devices=NCORES
    )
    xT = nc.dram_tensor("xT", [B, KC, 128, T], BF16, kind="ExternalInput")
    wq = nc.dram_tensor("wq", [KC, 128, DQ], BF16, kind="ExternalInput")
    wkv = nc.dram_tensor("wkv", [KC, 128, 128], BF16, kind="ExternalInput")
    wo = nc.dram_tensor("wo", [KC, 128, DQ], BF16, kind="ExternalInput")
    mw = 512 if causal else T
    maskT = nc.dram_tensor("maskT", [JC, 128, mw], BF16, kind="ExternalInput")
    ident = nc.dram_tensor("ident", [128, 128], BF16, kind="ExternalInput")
    outT = nc.dram_tensor("outT", [2, 128, NT], F32, kind="ExternalOutput")

    with tile.TileContext(nc) as tc, ExitStack() as top:
        wpool = top.enter_context(tc.tile_pool(name="weights", bufs=1))
        wq_sb = wpool.tile([128, KC, DQ], BF16)
        wkv_sb = wpool.tile([128, KC, 128], BF16)
        nc.gpsimd.dma_start(wq_sb[:], wq[:, :, :].rearrange("k p d -> p k d"))
        nc.gpsimd.dma_start(wkv_sb[:], wkv[:, :, :].rearrange("k p d -> p k d"))

        qkv_pool = top.enter_context(tc.tile_pool(name="qkv", bufs=1))
        qT = [qkv_pool.tile([64, NT], BF16, name=f"qT{h}") for h in range(QH)]
        kT = qkv_pool.tile([64, NT], BF16, name="kT")
        vT = qkv_pool.tile([64, NT], BF16, name="vT")
        vones = qkv_pool.tile([128, B, JC, DH + 1], BF16, name="vones")
        yT_sb = [qkv_pool.tile([128, NT], BF16, name=f"yTsb{i}") for i in range(2)]
        ident_sb = wpool.tile([128, 128], BF16, name="ident_sb")
        ones_sb = wpool.tile([1, 64], F32, name="ones_sb")
        nc.gpsimd.dma_start(ident_sb[:], ident[:])
        nc.vector.memset(ones_sb[:], 1.0)

        nc.vector.memset(vones[:, :, :, DH : DH + 1], 1.0)

        # ---------------- phase 1: QKV projections (transposed layout) ------
        with tc.tile_pool(name="xcol", bufs=2) as xpool, tc.tile_pool(
            name="qkvps", bufs=3, space="PSUM"
        ) as qkvps:
            for n in range(B * TC):
                b, nn = divmod(n, TC)
                col = slice(n * 512, (n + 1) * 512)
                xc = xpool.tile([128, KC, 512], BF16, name="xc")
                nc.gpsimd.dma_start(
                    xc[:],
                    xT[b, :, :, nn * 512 : (nn + 1) * 512].rearrange(
                        "k p t -> p k t"
                    ),
                )
                for m in range(2):  # q-head pairs (2m, 2m+1)
                    ps = qkvps.tile([128, 512], F32, name="ps")
                    for k in range(KC):
                        nc.tensor.matmul(
                            ps[:],
                            wq_sb[:, k, m * 128 : (m + 1) * 128],
                            xc[:, k, :],
                            start=(k == 0),
                            stop=(k == KC - 1),
                        )
                    nc.vector.tensor_copy(qT[2 * m][0:64, col], ps[0:64, :])
                    nc.vector.tensor_copy(qT[2 * m + 1][0:64, col], ps[64:128, :])
                ps = qkvps.tile([128, 512], F32, name="ps")
                for k in range(KC):
                    nc.tensor.matmul(
                        ps[:],
                        wkv_sb[:, k, :],
                        xc[:, k, :],
                        start=(k == 0),
                        stop=(k == KC - 1),
                    )
                nc.vector.tensor_copy(kT[0:64, col], ps[0:64, :])
                nc.vector.tensor_copy(vT[0:64, col], ps[64:128, :])

        # ---------------- phase 1.5: V to natural layout (PE transpose) -----
        with tc.tile_pool(name="tps", bufs=3, space="PSUM") as tpool:
            for b in range(B):
                for j in range(JC):
                    tp = tpool.tile([128, DH], BF16, name="tp")
                    nc.tensor.transpose(
                        tp[:],
                        vT[0:64, b * T + j * 128 : b * T + (j + 1) * 128],
                        ident_sb[0:64, 0:64],
                    )
                    nc.vector.tensor_copy(vones[:, b, j, 0:DH], tp[:])

        dpool = top.enter_context(tc.tile_pool(name="dram", bufs=1, space="DRAM"))
        yT_in_b = [dpool.tile([DQ, T], BF16, name=f"yTin{b}") for b in range(B)]
        yT_all_b = [
            dpool.tile([KC, 128, T], BF16, addr_space="Shared", name=f"yTall{b}")
            for b in range(B)
        ]

        # ---------------- phase 2: attention ------------------------------
        with tc.tile_pool(name="spool", bufs=1, space="PSUM") as spool, tc.tile_pool(
            name="ypool", bufs=1, space="PSUM"
        ) as ypsum, tc.tile_pool(name="ppool", bufs=3) as ppool, tc.tile_pool(
            name="mpool", bufs=1
        ) as mpool, tc.tile_pool(name="rpool", bufs=2) as rpool:
            mask_sb = mpool.tile([128, JC, mw], BF16, name="mask_sb")
            nc.gpsimd.dma_start(
                mask_sb[:], maskT[:, :, :].rearrange("j p w -> p j w")
            )
            for b in range(B):
                for h in range(QH):
                    y_acc = ypsum.tile([DH + 1, T], F32, name="y_acc")
                    for j in range(JC):
                        q0 = 512 * (j // 4) if causal else 0
                        w = T - q0
                        S = spool.tile([128, T], F32, name="S")
                        lk = kT[0:64, b * T + j * 128 : b * T + (j + 1) * 128]
                        for u in range(w // 512):
                            nc.tensor.matmul(
                                S[:, u * 512 : (u + 1) * 512],
                                lk,
                                qT[h][
                                    0:64,
                                    b * T + q0 + u * 512 : b * T + q0 + (u + 1) * 512,
                                ],
                                start=True,
                                stop=True,
                            )
                        nc.vector.tensor_add(
                            S[:, 0:mw], S[:, 0:mw], mask_sb[:, j, :]
                        )
                        pT = ppool.tile([128, T], BF16, name="pT")
                        nc.scalar.activation(pT[:, 0:w], S[:, 0:w], EXP)
                        for n2 in range(q0 // 512, TC):
                            last_j = 4 * n2 + 3 if causal else JC - 1
                            nc.tensor.matmul(
                                y_acc[:, n2 * 512 : (n2 + 1) * 512],
                                vones[:, b, j, :],
                                pT[:, n2 * 512 - q0 : n2 * 512 - q0 + 512],
                                start=(j == 0),
                                stop=(j == last_j),
                            )
                    r_sb = rpool.tile([1, T], F32, name="r_sb")
                    nc.vector.reciprocal(r_sb[:], y_acc[DH : DH + 1, :])
                    rb_ps = spool.tile([64, T], F32, name="rb_ps", tag="S")
                    for u in range(TC):
                        nc.tensor.matmul(
                            rb_ps[:, u * 512 : (u + 1) * 512],
                            ones_sb[:],
                            r_sb[:, u * 512 : (u + 1) * 512],
                            start=True,
                            stop=True,
                        )
                    rb = rpool.tile([64, T], F32, name="rb")
                    nc.vector.tensor_copy(rb[:], rb_ps[:])
                    dst = yT_sb[h // 2][
                        64 * (h % 2) : 64 * (h % 2) + 64, b * T : (b + 1) * T
                    ]
                    nc.vector.tensor_mul(dst, y_acc[0:DH, :], rb[:])
                nc.gpsimd.dma_start(
                    yT_in_b[b][0:128, :], yT_sb[0][:, b * T : (b + 1) * T]
                )
                nc.gpsimd.dma_start(
                    yT_in_b[b][128:256, :], yT_sb[1][:, b * T : (b + 1) * T]
                )
                nc.gpsimd.collective_compute(
                    "AllGather",
                    mybir.AluOpType.bypass,
                    replica_groups=[list(range(NCORES))],
                    ins=[yT_in_b[b].opt()],
                    outs=[yT_all_b[b].opt()],
                )

        # ------------- phase 4: output projection (out^T shard) --------
        with tc.tile_pool(name="ysl", bufs=3) as ylp, tc.tile_pool(
            name="wopool", bufs=1
        ) as wop, tc.tile_pool(
            name="popool", bufs=8, space="PSUM"
        ) as pop, tc.tile_pool(name="osb", bufs=2) as osp:
            wo_sb = wop.tile([128, KC, DQ], BF16, name="wo_sb")
            for k in range(KC):
                nc.gpsimd.dma_start(wo_sb[:, k, :], wo[k])
            for half in range(2):
                pos = [
                    [pop.tile([128, 512], F32, name="po") for _ in range(4)]
                    for _ in range(2)
                ]
                for k in range(KC):
                    ysl = ylp.tile([128, 2048], BF16, name="ysl")
                    nc.gpsimd.dma_start(ysl[:], yT_all_b[half][k, :, :])
                    for m in range(2):
                        for u in range(4):
                            nc.tensor.matmul(
                                pos[m][u][:],
                                wo_sb[:, k, m * 128 : (m + 1) * 128],
                                ysl[:, u * 512 : (u + 1) * 512],
                                start=(k == 0),
                                stop=(k == KC - 1),
                            )
                for m in range(2):
                    for u in range(4):
                        osb = osp.tile([128, 512], F32, name="osb")
                        nc.vector.tensor_copy(osb[:], pos[m][u][:])
                        nc.gpsimd.dma_start(
                            outT[
                                m,
                                :,
                                half * 2048 + u * 512 : half * 2048 + (u + 1) * 512,
                            ],
                            osb[:],
                        )
    if compile:
        nc.compile()
    return nc


_CACHE: dict = {}


def _get_compiled(causal: bool) -> bacc.Bacc:
    if causal not in _CACHE:
        _CACHE[causal] = _build_fast() if causal else _build_ref(causal)
    return _CACHE[causal]


def _prep_common(x, Wq, Wk, Wv, Wo):
    x = np.asarray(x, dtype=np.float32)
    Wq = np.asarray(Wq, dtype=np.float32) * 0.125  # fold 1/sqrt(64) into Wq
    Wk = np.asarray(Wk, dtype=np.float32)
    Wv = np.asarray(Wv, dtype=np.float32)
    Wo = np.asarray(Wo, dtype=np.float32)

    xT = (
        np.ascontiguousarray(x.transpose(0, 2, 1))
        .reshape(B, KC, 128, T)
        .astype(NPBF16)
    )
    per_core = []
    for c in range(NCORES):
        wq_c = np.ascontiguousarray(Wq[:, c * DQ : (c + 1) * DQ]).reshape(
            KC, 128, DQ
        ).astype(NPBF16)
        wkv_c = np.concatenate(
            [Wk[:, c * DH : (c + 1) * DH], Wv[:, c * DH : (c + 1) * DH]], axis=1
        ).reshape(KC, 128, 128).astype(NPBF16)
        wo_c = np.ascontiguousarray(Wo[:, c * DQ : (c + 1) * DQ]).reshape(
            KC, 128, DQ
        ).astype(NPBF16)
        per_core.append((wq_c, wkv_c, wo_c))
    return xT, per_core


def _prep_inputs_fast(x, attn_mask, Wq, Wk, Wv, Wo):
    xT, per_core = _prep_common(x, Wq, Wk, Wv, Wo)
    # S[key_local, q_local] += tri[key_local, q_local]: -1e9 where key > q.
    tri = np.tril(np.full((128, 128), -1e9, dtype=np.float32), k=-1).astype(
        NPBF16
    )
    ident = np.eye(128, dtype=NPBF16)
    return [
        {
            "xT": xT,
            "wq": wq_c,
            "wkv": wkv_c,
            "wo": wo_c,
            "tri": tri,
            "ident": ident,
        }
        for (wq_c, wkv_c, wo_c) in per_core
    ]


def _prep_inputs_ref(x, attn_mask, Wq, Wk, Wv, Wo, causal):
    xT, per_core = _prep_common(x, Wq, Wk, Wv, Wo)
    mask2d = np.asarray(attn_mask, dtype=np.float32).reshape(T, T)
    if causal:
        maskT = np.stack(
            [
                mask2d[
                    512 * (j // 4) : 512 * (j // 4) + 512, 128 * j : 128 * (j + 1)
                ].T
                for j in range(JC)
            ]
        ).astype(NPBF16)
    else:
        maskT = np.stack(
            [mask2d[:, 128 * j : 128 * (j + 1)].T for j in range(JC)]
        ).astype(NPBF16)
    ident = np.eye(128, dtype=NPBF16)
    return [
        {
            "xT": xT,
            "wq": wq_c,
            "wkv": wkv_c,
            "wo": wo_c,
            "maskT": maskT,
            "ident": ident,
        }
        for (wq_c, wkv_c, wo_c) in per_core
    ]


def _is_causal(attn_mask) -> bool:
    mask2d = np.asarray(attn_mask, dtype=np.float32).reshape(T, T)
    ref = np.triu(np.full((T, T), -1e9, dtype=np.float32), k=1)
    return bool(np.array_equal(mask2d, ref))


def _run(x, attn_mask, Wq, Wk, Wv, Wo, trace=False, trace_cores=None):
    causal = _is_causal(attn_mask)
    nc = _get_compiled(causal)
    if causal:
        in_maps = _prep_inputs_fast(x, attn_mask, Wq, Wk, Wv, Wo)
    else:
        in_maps = _prep_inputs_ref(x, attn_mask, Wq, Wk, Wv, Wo, causal)
    res = run_bass_kernel_spmd(
        nc,
        in_maps,
        core_ids=list(range(NCORES)),
        trace=trace,
        trace_cores=trace_cores,
    )
    outT = np.concatenate(
        [np.asarray(r["outT"], dtype=np.float32).reshape(DQ, NT) for r in res.results],
        axis=0,
    )
    out = np.ascontiguousarray(outT.T).reshape(B, T, HID).astype(np.float32)
    return out, res


def kernel(x, attn_mask, Wq, Wk, Wv, Wo):
    out, _ = _run(x, attn_mask, Wq, Wk, Wv, Wo)
    return out


if __name__ == "__main__":
    rng = np.random.default_rng(0)
    x = rng.standard_normal((B, T, HID), dtype=np.float32)
    mask = np.triu(np.full((T, T), -1e9, dtype=np.float32), k=1)[None, None]
    s = 1.0 / np.sqrt(HID)
    Wq = rng.standard_normal((HID, HEADS * DH), dtype=np.float32) * s
    Wk = rng.standard_normal((HID, KV_HEADS * DH), dtype=np.float32) * s
    Wv = rng.standard_normal((HID, KV_HEADS * DH), dtype=np.float32) * s
    Wo = rng.standard_normal((HEADS * DH, HID), dtype=np.float32) * s
    out = kernel(x, mask, Wq, Wk, Wv, Wo)
    print("out", out.shape, out.dtype, np.abs(out).mean())


# revision 9
# speedup vs baseline: 1.9987x; 1.1601x over previous
"""GQA attention (B=2, T=2048, HID=2048, 32 q-heads / 8 kv-heads, d=64)
distributed over 8 TRN2 NeuronCores.

Sharding: tensor-parallel over heads. Core c owns q-heads [4c, 4c+4) and
kv-head c (column shards of Wq/Wk/Wv), plus the matching column shard of Wo
used to compute out^T rows. x is replicated (host pre-transposes to [hid, tok]
and casts to bf16). After local attention each core AllGathers its y^T
[256, 4096] block into the full y^T [2048, 4096], then computes
out^T[256c:256c+256, :] locally. The host concatenates and transposes.

Fast causal path design notes:
- PE p-states: the PE only reaches 2.4 GHz after ~3us of continuous busy.
  The attention j-loop is software-pipelined (scores run 2 iterations ahead
  of PV) with triple-buffered S/pT tiles so the PE never stalls on the
  exp (ACT engine) round-trip.
- Softmax denominator: V is augmented with 64 columns of ones, so the PV
  matmul itself produces the denominator replicated across 64 PSUM
  partitions (matmul cost depends only on N, not M). Normalization is then
  one DVE reciprocal [64,512] + one DVE multiply per (head, q-block).
- Causal trim: scores/exp are only computed for the live region at
  128-column granularity; diagonal-block pT tiles keep a pre-zeroed
  masked prefix so PV can run untrimmed with correct start/stop flags.
- Heads processed in sweeps of 2 so PSUM fits: S pair tiles [128,2,512]
  x3 bufs (6 banks) + 2 y_acc [128,512] (2 banks) = 8 banks.

All matmuls run in bf16 with f32 PSUM accumulation. Softmax runs without
max-subtraction (scores are O(10) for this distribution; exp is exact in
f32) and exp(-1e9) underflows to exactly 0 for the masked region.
"""

import os
import sys

import numpy as np

for _p in ("/opt/trn_rl_repo", "/root/.axon_site/_ro/trn_rl_repo"):
    if os.path.isdir(_p) and _p not in sys.path:
        sys.path.append(_p)

import ml_dtypes  # noqa: E402
from contextlib import ExitStack  # noqa: E402

import concourse.bass as bass  # noqa: E402
import concourse.tile as tile  # noqa: E402
from concourse import bacc, mybir  # noqa: E402
from concourse.bass_utils import run_bass_kernel_spmd  # noqa: E402

BF16 = mybir.dt.bfloat16
F32 = mybir.dt.float32
NPBF16 = ml_dtypes.bfloat16

B, T, HID = 2, 2048, 2048
NT = B * T
HEADS, KV_HEADS, DH = 32, 8, 64
NCORES = 8
QH = HEADS // NCORES          # q-heads per core
DQ = QH * DH                  # 256
KC = HID // 128               # 16 hidden-dim chunks
TC = T // 512                 # 4 token chunks of 512 per batch
JC = T // 128                 # 16 key chunks of 128 per batch
EXP = mybir.ActivationFunctionType.Exp


LN = mybir.ActivationFunctionType.Ln


def _build_fast(debug: bool = False, compile: bool = True) -> bacc.Bacc:
    """Fast builder for the causal mask case."""
    nc = bacc.Bacc(
        "TRN2", target_bir_lowering=False, debug=debug, num_devices=NCORES
    )
    xT = nc.dram_tensor("xT", [B, KC, 128, T], BF16, kind="ExternalInput")
    wq = nc.dram_tensor("wq", [KC, 128, DQ], BF16, kind="ExternalInput")
    wkv = nc.dram_tensor("wkv", [KC, 128, 128], BF16, kind="ExternalInput")
    wo = nc.dram_tensor("wo", [KC, 128, DQ], BF16, kind="ExternalInput")
    tri = nc.dram_tensor("tri", [128, 128], BF16, kind="ExternalInput")
    ident = nc.dram_tensor("ident", [128, 128], BF16, kind="ExternalInput")
    outT = nc.dram_tensor("outT", [2, 128, NT], F32, kind="ExternalOutput")

    with tile.TileContext(nc) as tc, ExitStack() as top:
        wpool = top.enter_context(tc.tile_pool(name="weights", bufs=1))
        wq_sb = wpool.tile([128, KC, DQ], BF16)
        wkv_sb = wpool.tile([128, KC, 128], BF16)
        wo_sb = wpool.tile([128, KC, DQ], BF16)
        ident_sb = wpool.tile([128, 128], BF16, name="ident_sb")
        tri_sb = wpool.tile([128, 128], BF16, name="tri_sb")
        nc.gpsimd.dma_start(ident_sb[:], ident[:])
        nc.gpsimd.dma_start(tri_sb[:], tri[:])
        nc.gpsimd.dma_start(wq_sb[:], wq[:, :, :].rearrange("k p d -> p k d"))
        nc.gpsimd.dma_start(wkv_sb[:], wkv[:, :, :].rearrange("k p d -> p k d"))
        nc.gpsimd.dma_start(wo_sb[:], wo[:, :, :].rearrange("k p d -> p k d"))

        qkv_pool = top.enter_context(tc.tile_pool(name="qkv", bufs=1))
        # q stored as head pairs [64, 2, NT] so one scores matmul can stream
        # both heads of a sweep (N = 2*512).
        qTp = [qkv_pool.tile([64, 2, NT], BF16, name=f"qTp{m}") for m in range(2)]
        kT = qkv_pool.tile([64, NT], BF16, name="kT")
        vT = qkv_pool.tile([64, NT], BF16, name="vT")
        # V augmented with 64 ones-columns: PV matmul then emits the softmax
        # denominator replicated on PSUM partitions 64..127.
        vones = qkv_pool.tile([128, B, JC, 128], BF16, name="vones")
        yT_sb = [qkv_pool.tile([128, NT], BF16, name=f"yTsb{i}") for i in range(2)]
        nc.vector.memset(vones[:, :, :, DH:128], 1.0)

        # pT pools: rotating tiles for off-diagonal blocks, plus one
        # dedicated tile per diagonal offset r with a permanently-zero
        # masked prefix (exp never writes columns < 128r).
        pt_pool = top.enter_context(tc.tile_pool(name="ptf", bufs=3))
        ptd_pool = top.enter_context(tc.tile_pool(name="ptd", bufs=1))
        ptd = [
            ptd_pool.tile([128, 2, 512], BF16, name=f"ptd{r}") for r in range(4)
        ]
        for r in range(1, 4):
            nc.vector.memset(ptd[r][:, :, 0 : 128 * r], 0.0)

        # DRAM bounce buffers for the per-(batch, head-pair) AllGathers
        # (issued as soon as that half of y^T is done, so all but the last
        # overlap compute). SWDGE DMAs only near collectives — see
        # test_sync_dma_collective_hang.
        dpool = top.enter_context(tc.tile_pool(name="dram", bufs=1, space="DRAM"))
        yT_in = [
            [dpool.tile([128, T], BF16, name=f"yTin{b}_{hg}") for hg in range(2)]
            for b in range(B)
        ]
        yT_all = [
            [
                dpool.tile(
                    [NCORES, 128, T],
                    BF16,
                    addr_space="Shared",
                    name=f"yTall{b}_{hg}",
                )
                for hg in range(2)
            ]
            for b in range(B)
        ]

        # ---------------- phase 1: QKV projections (both batches) -----------
        with tc.tile_pool(name="xcol", bufs=2) as xpool, tc.tile_pool(
            name="qkvps", bufs=3, space="PSUM"
        ) as qkvps, tc.tile_pool(name="tps", bufs=2, space="PSUM") as tpool:
            for n in range(NT // 512):
                col = slice(n * 512, (n + 1) * 512)
                b, nn = divmod(n, TC)
                xc = xpool.tile([128, KC, 512], BF16, name="xc")
                # split the load so the first matmuls start early
                for kq in range(4):
                    nc.gpsimd.dma_start(
                        xc[:, kq * 4 : (kq + 1) * 4, :],
                        xT[
                            b, kq * 4 : (kq + 1) * 4, :, nn * 512 : (nn + 1) * 512
                        ].rearrange("k p t -> p k t"),
                    )
                for m in range(2):  # q-head pairs (2m, 2m+1)
                    ps = qkvps.tile([128, 512], F32, name="ps")
                    for k in range(KC):
                        nc.tensor.matmul(
                            ps[:],
                            wq_sb[:, k, m * 128 : (m + 1) * 128],
                            xc[:, k, :],
                            start=(k == 0),
                            stop=(k == KC - 1),
                        )
                    nc.vector.tensor_copy(qTp[m][0:64, 0, col], ps[0:64, :])
                    nc.vector.tensor_copy(qTp[m][0:64, 1, col], ps[64:128, :])
                ps = qkvps.tile([128, 512], F32, name="ps")
                for k in range(KC):
                    nc.tensor.matmul(
                        ps[:],
                        wkv_sb[:, k, :],
                        xc[:, k, :],
                        start=(k == 0),
                        stop=(k == KC - 1),
                    )
                nc.vector.tensor_copy(kT[0:64, col], ps[0:64, :])
                nc.vector.tensor_copy(vT[0:64, col], ps[64:128, :])

            # V to natural layout (PE transpose)
            for b in range(B):
                for j in range(JC):
                    tp = tpool.tile([128, DH], BF16, name="tp")
                    nc.tensor.transpose(
                        tp[:],
                        vT[0:64, b * T + j * 128 : b * T + (j + 1) * 128],
                        ident_sb[0:64, 0:64],
                    )
                    nc.vector.tensor_copy(vones[:, b, j, 0:DH], tp[:])

        # ---------------- phase 2: attention --------------------------------
        with tc.tile_pool(name="spool", bufs=2, space="PSUM") as spool, tc.tile_pool(
            name="ypool", bufs=2, space="PSUM"
        ) as ypool, tc.tile_pool(name="rpool", bufs=2) as rpool:
            for b in range(B):
                for hg in range(2):  # head pair (2hg, 2hg+1)
                    for n2 in range(TC):
                        y_acc = ypool.tile([128, 2, 512], F32, name="ya")
                        nj = 4 * (n2 + 1)
                        qb = b * T + n2 * 512
                        pend = []  # software pipeline: PV lags scores by 2

                        def emit_pv(ent):
                            pj, ppt = ent
                            for hh in range(2):
                                nc.tensor.matmul(
                                    y_acc[:, hh, :],
                                    vones[:, b, pj, :],
                                    ppt[:, hh, :],
                                    start=(pj == 0),
                                    stop=(pj == nj - 1),
                                )

                        for j in range(nj):
                            diag = j >= 4 * n2
                            off = 128 * (j - 4 * n2) if diag else 0
                            S = spool.tile([128, 2, 512], F32, name="S")
                            lk = kT[0:64, b * T + j * 128 : b * T + (j + 1) * 128]
                            for hh in range(2):
                                nc.tensor.matmul(
                                    S[:, hh, off:512],
                                    lk,
                                    qTp[hg][0:64, hh, qb + off : qb + 512],
                                    start=True,
                                    stop=True,
                                )
                            if diag:
                                for hh in range(2):
                                    nc.vector.tensor_add(
                                        S[:, hh, off : off + 128],
                                        S[:, hh, off : off + 128],
                                        tri_sb[:],
                                    )
                                pt = ptd[j - 4 * n2]
                            else:
                                pt = pt_pool.tile([128, 2, 512], BF16, name="pt")
                            nc.scalar.activation(
                                pt[:, :, off:512], S[:, :, off:512], EXP
                            )
                            pend.append((j, pt))
                            if len(pend) > 2:
                                emit_pv(pend.pop(0))
                        for ent in pend:
                            emit_pv(ent)

                        # normalize on DVE (y_acc is double-buffered, so this
                        # drains off the critical path)
                        for hh in range(2):
                            rec = rpool.tile([64, 512], F32, name="rec")
                            nc.vector.reciprocal(rec[:], y_acc[64:128, hh, :])
                            nc.vector.tensor_mul(
                                yT_sb[hg][
                                    64 * hh : 64 * hh + 64, qb : qb + 512
                                ],
                                y_acc[0:64, hh, :],
                                rec[:],
                            )

                    # AllGather this (batch, head-pair) block of y^T
                    nc.gpsimd.dma_start(
                        yT_in[b][hg][:], yT_sb[hg][:, b * T : (b + 1) * T]
                    )
                    nc.gpsimd.collective_compute(
                        "AllGather",
                        mybir.AluOpType.bypass,
                        replica_groups=[list(range(NCORES))],
                        ins=[yT_in[b][hg].opt()],
                        outs=[yT_all[b][hg].opt()],
                    )

        # ---------------- phase 3: output projection (out^T shard) ----------
        # y^T chunk k (128 rows) comes from core k//2, head-pair k%2; consume
        # head-pair-0 chunks first so each half only waits on its first AG.
        with tc.tile_pool(name="ysl", bufs=4) as ylp, tc.tile_pool(
            name="popool", bufs=1, space="PSUM"
        ) as pop, tc.tile_pool(name="osb", bufs=2) as osp:
            korder = [2 * c for c in range(NCORES)] + [
                2 * c + 1 for c in range(NCORES)
            ]
            for half in range(2):
                pos = [
                    [pop.tile([128, 512], F32, name=f"po{m}{u}") for u in range(4)]
                    for m in range(2)
                ]
                for ki, k in enumerate(korder):
                    ysl = ylp.tile([128, 2048], BF16, name="ysl")
                    nc.gpsimd.dma_start(ysl[:], yT_all[half][k % 2][k // 2, :, :])
                    for m in range(2):
                        for u in range(4):
                            nc.tensor.matmul(
                                pos[m][u][:],
                                wo_sb[:, k, m * 128 : (m + 1) * 128],
                                ysl[:, u * 512 : (u + 1) * 512],
                                start=(ki == 0),
                                stop=(ki == KC - 1),
                            )
                for m in range(2):
                    for u in range(4):
                        osb = osp.tile([128, 512], F32, name="osb")
                        nc.vector.tensor_copy(osb[:], pos[m][u][:])
                        nc.gpsimd.dma_start(
                            outT[
                                m,
                                :,
                                half * 2048 + u * 512 : half * 2048 + (u + 1) * 512,
                            ],
                            osb[:],
                        )
    if compile:
        nc.compile()
    return nc


def _build_ref(causal: bool, debug: bool = False, compile: bool = True) -> bacc.Bacc:
    """Reference builder (handles arbitrary masks; used for non-causal)."""
    nc = bacc.Bacc(
        "TRN2", target_bir_lowering=False, debug=debug, num_devices=NCORES
    )
    xT = nc.dram_tensor("xT", [B, KC, 128, T], BF16, kind="ExternalInput")
    wq = nc.dram_tensor("wq", [KC, 128, DQ], BF16, kind="ExternalInput")
    wkv = nc.dram_tensor("wkv", [KC, 128, 128], BF16, kind="ExternalInput")
    wo = nc.dram_tensor("wo", [KC, 128, DQ], BF16, kind="ExternalInput")
    mw = 512 if causal else T
    maskT = nc.dram_tensor("maskT", [JC, 128, mw], BF16, kind="ExternalInput")
    ident = nc.dram_tensor("ident", [128, 128], BF16, kind="ExternalInput")
    outT = nc.dram_tensor("outT", [2, 128, NT], F32, kind="ExternalOutput")

    with tile.TileContext(nc) as tc, ExitStack() as top:
        wpool = top.enter_context(tc.tile_pool(name="weights", bufs=1))
        wq_sb = wpool.tile([128, KC, DQ], BF16)
        wkv_sb = wpool.tile([128, KC, 128], BF16)
        nc.gpsimd.dma_start(wq_sb[:], wq[:, :, :].rearrange("k p d -> p k d"))
        nc.gpsimd.dma_start(wkv_sb[:], wkv[:, :, :].rearrange("k p d -> p k d"))

        qkv_pool = top.enter_context(tc.tile_pool(name="qkv", bufs=1))
        qT = [qkv_pool.tile([64, NT], BF16, name=f"qT{h}") for h in range(QH)]
        kT = qkv_pool.tile([64, NT], BF16, name="kT")
        vT = qkv_pool.tile([64, NT], BF16, name="vT")
        vones = qkv_pool.tile([128, B, JC, DH + 1], BF16, name="vones")
        yT_sb = [qkv_pool.tile([128, NT], BF16, name=f"yTsb{i}") for i in range(2)]
        ident_sb = wpool.tile([128, 128], BF16, name="ident_sb")
        ones_sb = wpool.tile([1, 64], F32, name="ones_sb")
        nc.gpsimd.dma_start(ident_sb[:], ident[:])
        nc.vector.memset(ones_sb[:], 1.0)

        nc.vector.memset(vones[:, :, :, DH : DH + 1], 1.0)

        # ---------------- phase 1: QKV projections (transposed layout) ------
        with tc.tile_pool(name="xcol", bufs=2) as xpool, tc.tile_pool(
            name="qkvps", bufs=3, space="PSUM"
        ) as qkvps:
            for n in range(B * TC):
                b, nn = divmod(n, TC)
                col = slice(n * 512, (n + 1) * 512)
                xc = xpool.tile([128, KC, 512], BF16, name="xc")
                nc.gpsimd.dma_start(
                    xc[:],
                    xT[b, :, :, nn * 512 : (nn + 1) * 512].rearrange(
                        "k p t -> p k t"
                    ),
                )
                for m in range(2):  # q-head pairs (2m, 2m+1)
                    ps = qkvps.tile([128, 512], F32, name="ps")
                    for k in range(KC):
                        nc.tensor.matmul(
                            ps[:],
                            wq_sb[:, k, m * 128 : (m + 1) * 128],
                            xc[:, k, :],
                            start=(k == 0),
                            stop=(k == KC - 1),
                        )
                    nc.vector.tensor_copy(qT[2 * m][0:64, col], ps[0:64, :])
                    nc.vector.tensor_copy(qT[2 * m + 1][0:64, col], ps[64:128, :])
                ps = qkvps.tile([128, 512], F32, name="ps")
                for k in range(KC):
                    nc.tensor.matmul(
                        ps[:],
                        wkv_sb[:, k, :],
                        xc[:, k, :],
                        start=(k == 0),
                        stop=(k == KC - 1),
                    )
                nc.vector.tensor_copy(kT[0:64, col], ps[0:64, :])
                nc.vector.tensor_copy(vT[0:64, col], ps[64:128, :])

        # ---------------- phase 1.5: V to natural layout (PE transpose) -----
        with tc.tile_pool(name="tps", bufs=3, space="PSUM") as tpool:
            for b in range(B):
                for j in range(JC):
                    tp = tpool.tile([128, DH], BF16, name="tp")
                    nc.tensor.transpose(
                        tp[:],
                        vT[0:64, b * T + j * 128 : b * T + (j + 1) * 128],
                        ident_sb[0:64, 0:64],
                    )
                    nc.vector.tensor_copy(vones[:, b, j, 0:DH], tp[:])

        dpool = top.enter_context(tc.tile_pool(name="dram", bufs=1, space="DRAM"))
        yT_in_b = [dpool.tile([DQ, T], BF16, name=f"yTin{b}") for b in range(B)]
        yT_all_b = [
            dpool.tile([KC, 128, T], BF16, addr_space="Shared", name=f"yTall{b}")
            for b in range(B)
        ]

        # ---------------- phase 2: attention ------------------------------
        with tc.tile_pool(name="spool", bufs=1, space="PSUM") as spool, tc.tile_pool(
            name="ypool", bufs=1, space="PSUM"
        ) as ypsum, tc.tile_pool(name="ppool", bufs=3) as ppool, tc.tile_pool(
            name="mpool", bufs=1
        ) as mpool, tc.tile_pool(name="rpool", bufs=2) as rpool:
            mask_sb = mpool.tile([128, JC, mw], BF16, name="mask_sb")
            nc.gpsimd.dma_start(
                mask_sb[:], maskT[:, :, :].rearrange("j p w -> p j w")
            )
            for b in range(B):
                for h in range(QH):
                    y_acc = ypsum.tile([DH + 1, T], F32, name="y_acc")
                    for j in range(JC):
                        q0 = 512 * (j // 4) if causal else 0
                        w = T - q0
                        S = spool.tile([128, T], F32, name="S")
                        lk = kT[0:64, b * T + j * 128 : b * T + (j + 1) * 128]
                        for u in range(w // 512):
                            nc.tensor.matmul(
                                S[:, u * 512 : (u + 1) * 512],
                                lk,
                                qT[h][
                                    0:64,
                                    b * T + q0 + u * 512 : b * T + q0 + (u + 1) * 512,
                                ],
                                start=True,
                                stop=True,
                            )
                        nc.vector.tensor_add(
                            S[:, 0:mw], S[:, 0:mw], mask_sb[:, j, :]
                        )
                        pT = ppool.tile([128, T], BF16, name="pT")
                        nc.scalar.activation(pT[:, 0:w], S[:, 0:w], EXP)
                        for n2 in range(q0 // 512, TC):
                            last_j = 4 * n2 + 3 if causal else JC - 1
                            nc.tensor.matmul(
                                y_acc[:, n2 * 512 : (n2 + 1) * 512],
                                vones[:, b, j, :],
                                pT[:, n2 * 512 - q0 : n2 * 512 - q0 + 512],
                                start=(j == 0),
                                stop=(j == last_j),
                            )
                    r_sb = rpool.tile([1, T], F32, name="r_sb")
                    nc.vector.reciprocal(r_sb[:], y_acc[DH : DH + 1, :])
                    rb_ps = spool.tile([64, T], F32, name="rb_ps", tag="S")
                    for u in range(TC):
                        nc.tensor.matmul(
                            rb_ps[:, u * 512 : (u + 1) * 512],
                            ones_sb[:],
                            r_sb[:, u * 512 : (u + 1) * 512],
                            start=True,
                            stop=True,
                        )
                    rb = rpool.tile([64, T], F32, name="rb")
                    nc.vector.tensor_copy(rb[:], rb_ps[:])
                    dst = yT_sb[h // 2][
                        64 * (h % 2) : 64 * (h % 2) + 64, b * T : (b + 1) * T
                    ]
                    nc.vector.tensor_mul(dst, y_acc[0:DH, :], rb[:])
                nc.gpsimd.dma_start(
                    yT_in_b[b][0:128, :], yT_sb[0][:, b * T : (b + 1) * T]
                )
                nc.gpsimd.dma_start(
                    yT_in_b[b][128:256, :], yT_sb[1][:, b * T : (b + 1) * T]
                )
                nc.gpsimd.collective_compute(
                    "AllGather",
                    mybir.AluOpType.bypass,
                    replica_groups=[list(range(NCORES))],
                    ins=[yT_in_b[b].opt()],
                    outs=[yT_all_b[b].opt()],
                )

        # ------------- phase 4: output projection (out^T shard) --------
        with tc.tile_pool(name="ysl", bufs=3) as ylp, tc.tile_pool(
            name="wopool", bufs=1
        ) as wop, tc.tile_pool(
            name="popool", bufs=8, space="PSUM"
        ) as pop, tc.tile_pool(name="osb", bufs=2) as osp:
            wo_sb = wop.tile([128, KC, DQ], BF16, name="wo_sb")
            for k in range(KC):
                nc.gpsimd.dma_start(wo_sb[:, k, :], wo[k])
            for half in range(2):
                pos = [
                    [pop.tile([128, 512], F32, name="po") for _ in range(4)]
                    for _ in range(2)
                ]
                for k in range(KC):
                    ysl = ylp.tile([128, 2048], BF16, name="ysl")
                    nc.gpsimd.dma_start(ysl[:], yT_all_b[half][k, :, :])
                    for m in range(2):
                        for u in range(4):
                            nc.tensor.matmul(
                                pos[m][u][:],
                                wo_sb[:, k, m * 128 : (m + 1) * 128],
                                ysl[:, u * 512 : (u + 1) * 512],
                                start=(k == 0),
                                stop=(k == KC - 1),
                            )
                for m in range(2):
                    for u in range(4):
                        osb = osp.tile([128, 512], F32, name="osb")
                        nc.vector.tensor_copy(osb[:], pos[m][u][:])
                        nc.gpsimd.dma_start(
                            outT[
                                m,
                                :,
                                half * 2048 + u * 512 : half * 2048 + (u + 1) * 512,
                            ],
                            osb[:],
                        )
    if compile:
        nc.compile()
    return nc


_CACHE: dict = {}


def _get_compiled(causal: bool) -> bacc.Bacc:
    if causal not in _CACHE:
        _CACHE[causal] = _build_fast() if causal else _build_ref(causal)
    return _CACHE[causal]


def _prep_common(x, Wq, Wk, Wv, Wo):
    x = np.asarray(x, dtype=np.float32)
    Wq = np.asarray(Wq, dtype=np.float32) * 0.125  # fold 1/sqrt(64) into Wq
    Wk = np.asarray(Wk, dtype=np.float32)
    Wv = np.asarray(Wv, dtype=np.float32)
    Wo = np.asarray(Wo, dtype=np.float32)

    xT = (
        np.ascontiguousarray(x.transpose(0, 2, 1))
        .reshape(B, KC, 128, T)
        .astype(NPBF16)
    )
    per_core = []
    for c in range(NCORES):
        wq_c = np.ascontiguousarray(Wq[:, c * DQ : (c + 1) * DQ]).reshape(
            KC, 128, DQ
        ).astype(NPBF16)
        wkv_c = np.concatenate(
            [Wk[:, c * DH : (c + 1) * DH], Wv[:, c * DH : (c + 1) * DH]], axis=1
        ).reshape(KC, 128, 128).astype(NPBF16)
        wo_c = np.ascontiguousarray(Wo[:, c * DQ : (c + 1) * DQ]).reshape(
            KC, 128, DQ
        ).astype(NPBF16)
        per_core.append((wq_c, wkv_c, wo_c))
    return xT, per_core


def _prep_inputs_fast(x, attn_mask, Wq, Wk, Wv, Wo):
    xT, per_core = _prep_common(x, Wq, Wk, Wv, Wo)
    # S[key_local, q_local] += tri[key_local, q_local]: -1e9 where key > q.
    tri = np.tril(np.full((128, 128), -1e9, dtype=np.float32), k=-1).astype(
        NPBF16
    )
    ident = np.eye(128, dtype=NPBF16)
    return [
        {
            "xT": xT,
            "wq": wq_c,
            "wkv": wkv_c,
            "wo": wo_c,
            "tri": tri,
            "ident": ident,
        }
        for (wq_c, wkv_c, wo_c) in per_core
    ]


def _prep_inputs_ref(x, attn_mask, Wq, Wk, Wv, Wo, causal):
    xT, per_core = _prep_common(x, Wq, Wk, Wv, Wo)
    mask2d = np.asarray(attn_mask, dtype=np.float32).reshape(T, T)
    if causal:
        maskT = np.stack(
            [
                mask2d[
                    512 * (j // 4) : 512 * (j // 4) + 512, 128 * j : 128 * (j + 1)
                ].T
                for j in range(JC)
            ]
        ).astype(NPBF16)
    else:
        maskT = np.stack(
            [mask2d[:, 128 * j : 128 * (j + 1)].T for j in range(JC)]
        ).astype(NPBF16)
    ident = np.eye(128, dtype=NPBF16)
    return [
        {
            "xT": xT,
            "wq": wq_c,
            "wkv": wkv_c,
            "wo": wo_c,
            "maskT": maskT,
            "ident": ident,
        }
        for (wq_c, wkv_c, wo_c) in per_core
    ]


def _is_causal(attn_mask) -> bool:
    mask2d = np.asarray(attn_mask, dtype=np.float32).reshape(T, T)
    ref = np.triu(np.full((T, T), -1e9, dtype=np.float32), k=1)
    return bool(np.array_equal(mask2d, ref))


def _run(x, attn_mask, Wq, Wk, Wv, Wo, trace=False, trace_cores=None):
    causal = _is_causal(attn_mask)
    nc = _get_compiled(causal)
    if causal:
        in_maps = _prep_inputs_fast(x, attn_mask, Wq, Wk, Wv, Wo)
    else:
        in_maps = _prep_inputs_ref(x, attn_mask, Wq, Wk, Wv, Wo, causal)
    res = run_bass_kernel_spmd(
        nc,
        in_maps,
        core_ids=list(range(NCORES)),
        trace=trace,
        trace_cores=trace_cores,
    )
    outT = np.concatenate(
        [np.asarray(r["outT"], dtype=np.float32).reshape(DQ, NT) for r in res.results],
        axis=0,
    )
    out = np.ascontiguousarray(outT.T).reshape(B, T, HID).astype(np.float32)
    return out, res


def kernel(x, attn_mask, Wq, Wk, Wv, Wo):
    out, _ = _run(x, attn_mask, Wq, Wk, Wv, Wo)
    return out


if __name__ == "__main__":
    rng = np.random.default_rng(0)
    x = rng.standard_normal((B, T, HID), dtype=np.float32)
    mask = np.triu(np.full((T, T), -1e9, dtype=np.float32), k=1)[None, None]
    s = 1.0 / np.sqrt(HID)
    Wq = rng.standard_normal((HID, HEADS * DH), dtype=np.float32) * s
    Wk = rng.standard_normal((HID, KV_HEADS * DH), dtype=np.float32) * s
    Wv = rng.standard_normal((HID, KV_HEADS * DH), dtype=np.float32) * s
    Wo = rng.standard_normal((HEADS * DH, HID), dtype=np.float32) * s
    out = kernel(x, mask, Wq, Wk, Wv, Wo)
    print("out", out.shape, out.dtype, np.abs(out).mean())
